# revision 30
# baseline (speedup 1.0000x reference)
"""MLA attention (DeepSeek-style) on 8 TRN2 NeuronCores — unabsorbed 2D
(batch-pair x head) sharding.

Sharding: cores (2b, 2b+1) own batch b. Core 2b runs heads 0-7, core 2b+1
heads 8-15, each over the full 1024 tokens of its batch. Preprocessing
(c_q, c_kv, roped k_r latents) is token-split within the pair (512 tokens
each) and exchanged with TWO pair-group AllGathers: c_kv right after it is
computed (so the exchange overlaps the c_q/k_r matmuls) and c_q+k_r after
phase 0 (overlapping the v_s/k_h matmuls at the start of attention).

Math: instead of the absorbed per-head k_eff = U_q @ U_k (making attention
contract over the 512-dim latent), we factor back to per-head q_h = c_q@U_q,
k_h = U_k@c_kv, v_h = c_kv@v_eff_h (HS=128-dim attention contraction) —
~2x fewer PE MACs, and AV output lands directly in the head's output block
(v_eff_h = (W_uv.T @ W_o.T) head-block keeps the output per-head local, so
no all-reduce).

Precision: bf16 everywhere on the PE (fp8-e4m3 DoubleRow was tried via the
FP8KV/FP8Q flags and REJECTED: rel-err 5.7e-2 > 2e-2 budget AND no speedup
— DoubleRow disables FWL so the halved column count is eaten by slower
weight loads); PSUM accumulation fp32. Attention is computed in the
transposed orientation logitsT[s, t] (no transposes anywhere; softmax
needs no max subtraction since logits are O(1)); column sums via an
appended ones-row matmul; the unnormalized y is written bf16; host divides
by the softmax denominators ([1, 8192] f32, one DMA).

Scheduling notes: DMA queue order is arranged so phase-0 inputs land
before the (large) resident attention weights (kills a ~16 us DMA lead-in
before the first matmul); q_r is issued before k_h/q_h on even heads so
the PE->DVE->Pool rope chain finishes under them; PSUM evacuations are
spread across ACT (exp, y th0, half the v_s/ckv) and DVE (projections,
y th1, sums) with masks on Pool, so the PE rarely waits on one engine.
Matmul N is capped at 512 (one PSUM bank) by ISA; per-MM overhead on HW
is ~125 ns (LDWEIGHTS largely unpipelined — walrus runs ldw-opt off), so
fewer-bigger matmuls and column-count cuts are the main levers. Measured:
removing the 96 sums matmuls (36864 cols) saves 16.7 us -> attention time
is ~proportional to PE columns at ~0.45 ns/col marginal.
"""

import math

import numpy as np

B, T, C = 4, 1024, 2048
NH, HS = 16, 128
NLQ = NLKV = 512
DHR = 64
NCORES = 8
HPC = 8                     # heads per core (half the heads, own batch)
TOK = 512                   # tokens per core in preprocessing (half a batch)
SCALE = 1.0 / math.sqrt(HS + DHR)
CC = C // 128               # 16 contraction chunks over C
FP8KV = False               # c_kv latent + U_k/v_eff in fp8-e4m3, DoubleRow
FP8Q = False                # c_q latent + U_q/W_qr in fp8-e4m3, DoubleRow

_cache = {}


def _build(loop_k=None, loop_pre=None, sim_single=False, phases="all",
           variant=""):
    """Build the SPMD kernel. loop_k / loop_pre: wrap the attention / the
    phase-0 body in a For_i hardware loop (timing amplification only).
    sim_single: single-core no-collective variant (gather outputs fed as
    inputs) for CoreSim/TimelineSim analysis. phases="pre" drops phase 2
    (cost-model phase attribution only)."""
    import contextlib

    import concourse.mybir as mybir
    import concourse.tile as tile
    from concourse import bacc

    f32 = mybir.dt.float32
    bf16 = mybir.dt.bfloat16
    fp8 = mybir.dt.float8e4
    kvdt = fp8 if FP8KV else bf16
    DR = mybir.MatmulPerfMode.DoubleRow
    Exp = mybir.ActivationFunctionType.Exp
    Copy = mybir.ActivationFunctionType.Copy
    mult = mybir.AluOpType.mult
    add = mybir.AluOpType.add

    nc = bacc.Bacc(trn_type="TRN2", num_devices=1 if sim_single else NCORES)
    P = nc.declare_dram_parameter

    xT = P("xT", [128, CC * TOK], bf16, isOutput=False)
    wdqT = P("wdqT", [128, CC * NLQ], bf16, isOutput=False)
    wdkvT = P("wdkvT", [128, CC * NLKV], bf16, isOutput=False)
    wkr2T = P("wkr2T", [128, CC * 2 * DHR], bf16, isOutput=False)
    wuqT = P("wuqT", [128, HPC * 512], fp8 if FP8Q else bf16, isOutput=False)
    wukT = P("wukT", [128, HPC * 512], kvdt, isOutput=False)
    wqr2T = P("wqr2T", [128, HPC * 1024 // 2], fp8 if FP8Q else bf16, isOutput=False)
    veffp = P("veffp", [128, 4 * HPC * HS], kvdt, isOutput=False)
    cos2d = P("cos2d", [128, T], f32, isOutput=False)
    sin2d = P("sin2d", [128, T], f32, isOutput=False)
    cos2o = P("cos2o", [DHR, TOK], f32, isOutput=False)
    sin2o = P("sin2o", [DHR, TOK], f32, isOutput=False)
    maskp = P("maskp", [128, 128], bf16, isOutput=False)
    out = P("out", [HPC * HS, T], bf16, isOutput=True)
    out2 = P("out2", [1, 16 * 512], f32, isOutput=True)
    ag1_p = ag2_p = ag3_p = None
    if sim_single:
        ag1_p = P("ag1_p", [2 * 128, 4 * TOK], kvdt, isOutput=False)
        ag2_p = P("ag2_p", [2 * 128, (4 if FP8Q else 5) * TOK],
                  fp8 if FP8Q else bf16, isOutput=False)
        if FP8Q:
            ag3_p = P("ag3_p", [2 * 128, TOK], bf16, isOutput=False)

    oldsums = "oldsums" in variant

    with tile.TileContext(nc) as tc:
        with (
            tc.tile_pool(name="pres", bufs=1) as pres,
            tc.tile_pool(name="dram", bufs=1, space="DRAM") as dram,
            # 8 PSUM banks total: work 2x[128,1024]=4, proj 2x[128,512]=2,
            # av 1, sums 1
            tc.tile_pool(name="ps_work", bufs=2, space="PSUM") as ps_work,
            tc.tile_pool(name="ps_proj", bufs=2, space="PSUM") as ps_proj,
            tc.tile_pool(name="ps_av", bufs=1, space="PSUM") as ps_av,
            tc.tile_pool(name="ps_sums", bufs=1, space="PSUM") as ps_sums,
        ):
            # ---------- resident tensors ----------
            wuqT_sb = (pres.tile([128, HPC * 4, 128], fp8, tag="wuqT",
                                 name="wuqT_sb")
                       if FP8Q else
                       pres.tile([128, HPC * 512], bf16, tag="wuqT",
                                 name="wuqT_sb"))
            wukT_sb = (pres.tile([128, HPC * 4, 128], fp8, tag="wukT", name="wukT_sb")
                       if FP8KV else
                       pres.tile([128, HPC * 512], bf16, tag="wukT", name="wukT_sb"))
            wqr2T_sb = (pres.tile([128, HPC * 4, 128], fp8, tag="wqr2T",
                                  name="wqr2T_sb")
                        if FP8Q else
                        pres.tile([128, HPC * 512], bf16, tag="wqr2T",
                                  name="wqr2T_sb"))
            cos2d_sb = pres.tile([128, T], f32, tag="cos2d")
            sin2d_sb = pres.tile([128, T], f32, tag="sin2d")
            cos2o_sb = pres.tile([DHR, TOK], f32, tag="cos2o")
            sin2o_sb = pres.tile([DHR, TOK], f32, tag="sin2o")
            v_eff_sb = (pres.tile([128, 4, HPC * HS], fp8, tag="v_eff", name="v_eff_sb")
                        if FP8KV else
                        pres.tile([128, 4 * HPC * HS], bf16, tag="v_eff", name="v_eff_sb"))
            cqT_f = (pres.tile([128, 8, 512], fp8, tag="cqT_f",
                               name="cqT_f")
                     if FP8Q else
                     pres.tile([128, 4 * T], bf16, tag="cqT_f",
                               name="cqT_f"))
            ckvT_f = (pres.tile([128, 8, 512], fp8, tag="ckvT_f", name="ckvT_f")
                      if FP8KV else
                      pres.tile([128, 4 * T], bf16, tag="ckvT_f", name="ckvT_f"))
            krT_f = pres.tile([DHR, T], bf16, tag="krT_f")
            ones_sb = pres.tile([128, 1], f32, tag="ones")
            ones_r = pres.tile([128, 1], bf16, tag="ones_r")
            mask_r = pres.tile([128, 128], bf16, tag="mask_r")

            nc.gpsimd.memset(ones_sb[:], 1.0)
            nc.vector.tensor_copy(ones_r[:], ones_sb[:])

            # DRAM bounce buffers for the pair AllGathers (Local output:
            # shared-output collectives need >4-core groups)
            agin1 = dram.tile([128, 4 * TOK], kvdt)    # c_kv
            agin2 = dram.tile([128, 5 * TOK], bf16)    # c_q ++ k_r
            agout1 = ag1_p if sim_single else dram.tile([2 * 128, 4 * TOK], kvdt)
            agout2 = ag2_p if sim_single else dram.tile([2 * 128, 5 * TOK], bf16)

            # ---------- phase 0: local latents (c_kv first) ----------
            def phase0():
              with (
                tc.tile_pool(name="p0", bufs=1) as p0,
                tc.For_i(0, loop_pre, 1, hint_engines=(mybir.EngineType.PE,))
                if loop_pre else contextlib.nullcontext(),
              ):
                xT_sb = p0.tile([128, CC * TOK], bf16, tag="xT")
                wdqT_sb = p0.tile([128, CC * NLQ], bf16, tag="wdqT")
                wdkvT_sb = p0.tile([128, CC * NLKV], bf16, tag="wdkvT")
                wkr2T_sb = p0.tile([128, CC * 2 * DHR], bf16, tag="wkr2T")
                cq_loc = p0.tile([128, 4 * TOK], fp8 if FP8Q else bf16,
                                 tag="cq_loc")
                ckv_loc = p0.tile([128, 4 * TOK], kvdt, tag="ckv_loc")
                kr_loc = p0.tile([DHR, TOK], bf16, tag="kr_loc")
                rtmp = p0.tile([DHR, 2 * TOK], f32, tag="rtmp")

                # phase-0 inputs first: xT on SP ring, wdkvT on ACT ring so
                # the first c_kv matmul can start after one chunk of each.
                for qr_ in range(4):
                    csl = slice(qr_ * 4 * TOK, (qr_ + 1) * 4 * TOK)
                    nc.sync.dma_start(xT_sb[:, csl], xT[:, csl])
                    nc.scalar.dma_start(wdkvT_sb[:, csl], wdkvT[:, csl])
                for qr_ in range(4):
                    wsl = slice(qr_ * 4 * NLQ, (qr_ + 1) * 4 * NLQ)
                    nc.sync.dma_start(wdqT_sb[:, wsl], wdqT[:, wsl])
                nc.scalar.dma_start(cos2o_sb[:], cos2o[:])
                nc.scalar.dma_start(sin2o_sb[:], sin2o[:])
                nc.scalar.dma_start(wkr2T_sb[:], wkr2T[:, :])
                if "p0dma" in variant:
                    # DMA-rate probe: consume the tiles with one trivial op
                    # so the loop body depends on every load, no compute.
                    nc.vector.tensor_copy(kr_loc[:, :1], xT_sb[:DHR, :1])
                    nc.vector.tensor_copy(kr_loc[:, 1:2], wdqT_sb[:DHR, :1])
                    nc.vector.tensor_copy(kr_loc[:, 2:3], wdkvT_sb[:DHR, :1])
                    nc.vector.tensor_copy(kr_loc[:, 3:4], wkr2T_sb[:DHR, :1])
                    return

                # c_kv: 4 PSUM slots held across cc-groups so compute on DMA
                # chunk g overlaps the chunk g+1 loads
                pks = [(ps_proj if kt % 2 else ps_work).tile(
                    [128, TOK], f32, tag="proj" if kt % 2 else "work",
                    name=f"p0ckv_{kt}")
                    for kt in range(4)]
                for g in range(4):
                    for kt in range(4):
                        for cc in range(4 * g, 4 * g + 4):
                            nc.tensor.matmul(
                                pks[kt][:],
                                wdkvT_sb[:, cc * NLKV + kt * 128: cc * NLKV + (kt + 1) * 128],
                                xT_sb[:, cc * TOK:(cc + 1) * TOK],
                                start=(cc == 0), stop=(cc == CC - 1))
                for kt in range(4):
                    eng = nc.vector if kt % 2 else nc.scalar
                    if kt % 2:
                        eng.tensor_copy(ckv_loc[:, kt * TOK:(kt + 1) * TOK],
                                        pks[kt][:])
                    else:
                        eng.activation(ckv_loc[:, kt * TOK:(kt + 1) * TOK],
                                       pks[kt][:], Copy)
                nc.gpsimd.dma_start(agin1[:], ckv_loc[:])

                # c_q
                pqs = [(ps_proj if qt % 2 else ps_work).tile(
                    [128, TOK], f32, tag="proj" if qt % 2 else "work",
                    name=f"p0cq_{qt}")
                    for qt in range(4)]
                for g in range(4):
                    for qt in range(4):
                        for cc in range(4 * g, 4 * g + 4):
                            nc.tensor.matmul(
                                pqs[qt][:],
                                wdqT_sb[:, cc * NLQ + qt * 128: cc * NLQ + (qt + 1) * 128],
                                xT_sb[:, cc * TOK:(cc + 1) * TOK],
                                start=(cc == 0), stop=(cc == CC - 1))
                for qt in range(4):
                    eng = nc.vector if qt % 2 else nc.scalar
                    if qt % 2:
                        eng.tensor_copy(cq_loc[:, qt * TOK:(qt + 1) * TOK],
                                        pqs[qt][:])
                    else:
                        eng.activation(cq_loc[:, qt * TOK:(qt + 1) * TOK],
                                       pqs[qt][:], Copy)

                # roped k_r: rows 0..63 raw, 64..127 pair-swapped copy
                pr = ps_proj.tile([128, TOK], f32, tag="proj")
                for cc in range(CC):
                    nc.tensor.matmul(
                        pr[:],
                        wkr2T_sb[:, cc * 2 * DHR:(cc + 1) * 2 * DHR],
                        xT_sb[:, cc * TOK:(cc + 1) * TOK],
                        start=(cc == 0), stop=(cc == CC - 1))
                nc.vector.tensor_tensor(rtmp[:, :TOK], pr[:DHR, :], cos2o_sb[:], mult)
                nc.vector.tensor_tensor(rtmp[:, TOK:], pr[DHR:, :], sin2o_sb[:], mult)
                nc.vector.tensor_tensor(kr_loc[:], rtmp[:, :TOK], rtmp[:, TOK:], add)

                if FP8Q:
                    nc.gpsimd.dma_start(agin2[:, :], cq_loc[:])
                    nc.gpsimd.dma_start(agin3[:DHR, :], kr_loc[:])
                    nc.gpsimd.dma_start(agin3[DHR:, :], kr_loc[:])
                else:
                    nc.gpsimd.dma_start(agin2[:, :4 * TOK], cq_loc[:])
                    nc.gpsimd.dma_start(agin2[:DHR, 4 * TOK:], kr_loc[:])
                    nc.gpsimd.dma_start(agin2[DHR:, 4 * TOK:], kr_loc[:])

            if phases != "attn":
                phase0()

            groups = [[2 * i, 2 * i + 1] for i in range(NCORES // 2)]
            if not sim_single and phases != "attn":
                nc.gpsimd.collective_compute(
                    "AllGather", mybir.AluOpType.bypass,
                    replica_groups=groups,
                    ins=[agin1.opt()], outs=[agout1.opt()])
                nc.gpsimd.collective_compute(
                    "AllGather", mybir.AluOpType.bypass,
                    replica_groups=groups,
                    ins=[agin2.opt()], outs=[agout2.opt()])
                if FP8Q:
                    nc.gpsimd.collective_compute(
                        "AllGather", mybir.AluOpType.bypass,
                        replica_groups=groups,
                        ins=[agin3.opt()], outs=[agout3.opt()])

            # resident attention weights, ordered by first use (after the
            # phase-0 inputs on both rings)
            nc.scalar.dma_start(v_eff_sb[:], veffp[:, :])
            nc.sync.dma_start(wukT_sb[:], wukT[:, :])
            nc.scalar.dma_start(wuqT_sb[:], wuqT[:, :])
            nc.sync.dma_start(wqr2T_sb[:], wqr2T[:, :])
            nc.sync.dma_start(cos2d_sb[:], cos2d[:])
            nc.scalar.dma_start(sin2d_sb[:], sin2d[:])
            nc.scalar.dma_start(mask_r[:], maskp[:])

            # ---------- phase 1: unpack gathered latents ----------
            # cqT_f/ckvT_f are r-major: col = ts*2048 + qc*512 + t_loc, so
            # each rank's block is ONE contiguous DMA from the gather output
            # split unpack across both HWDGE rings (SP + ACT) for parallelism
            ag1 = agout1.ap() if sim_single else agout1[:]
            ag2 = agout2.ap() if sim_single else agout2[:]
            ag3 = None
            if FP8Q:
                ag3 = agout3.ap() if sim_single else agout3[:]
            for r in range(2):
                rows = slice(r * 128, (r + 1) * 128)
                eng = nc.sync if r == 0 else nc.scalar
                if FP8KV:
                    eng.dma_start(ckvT_f[:, r * 4:(r + 1) * 4, :],
                                  ag1[rows, :])
                else:
                    eng.dma_start(
                        ckvT_f[:, r * 4 * TOK:(r + 1) * 4 * TOK],
                        ag1[rows, :])
                if FP8Q:
                    eng.dma_start(cqT_f[:, r * 4:(r + 1) * 4, :],
                                  ag2[rows, :])
                    eng.dma_start(
                        krT_f[:, r * TOK:(r + 1) * TOK],
                        ag3[r * 128: r * 128 + DHR, :])
                else:
                    eng.dma_start(
                        cqT_f[:, r * 4 * TOK:(r + 1) * 4 * TOK],
                        ag2[rows, :4 * TOK])
                    eng.dma_start(
                        krT_f[:, r * TOK:(r + 1) * TOK],
                        ag2[r * 128: r * 128 + DHR, 4 * TOK:])

            # ---------- phase 2: per-head projections + attention ----------
            with (
                tc.tile_pool(name="pv2", bufs=1) as pv2,
                tc.tile_pool(name="ph", bufs=2) as ph,
                tc.tile_pool(name="pqr", bufs=2) as pqr,
                tc.tile_pool(name="pex", bufs=14) as pex,
                tc.tile_pool(name="py", bufs=2) as py,
                tc.For_i(0, loop_k, 1, hint_engines=(mybir.EngineType.PE,))
                if loop_k else contextlib.nullcontext(),
            ):
                hpc_eff = 0 if phases == "pre" else HPC
                for tok in variant.split(","):
                    if tok.startswith("h") and tok[1:].isdigit():
                        hpc_eff = int(tok[1:])
                D2 = HPC * HS
                # softmax denominators: per head one PSUM bank with th=0 at
                # partition 0 and th=1 at partition 64 (both groups
                # accumulate concurrently); all 16 rows gathered in SBUF and
                # written out once
                if hpc_eff and not oldsums:
                    sums_sb = pv2.tile([1, 16 * 512], f32, tag="sums_sb")
                # v_s[s-chunk, d] for all 8 heads: [128, 8sc x (8h x 128d)]
                if hpc_eff:
                    v_s = pv2.tile([128, 8 * HPC * HS], bf16, tag="v_s")
                for sc in range(8 if hpc_eff else 0):
                    scb = (sc // 4) * 2048 + (sc % 4) * 128
                    pv = ps_work.tile([128, T], f32, tag="work")
                    # kc outer so the stationary ckv chunk is loaded once
                    # per (sc, kc) and reused by both nn matmuls
                    ts_, s_off = sc // 4, (sc % 4) * 128
                    for kc in range(2 if FP8KV else 4):
                        for nn in range(2):
                            if FP8KV:
                                nc.tensor.matmul(
                                    pv[:, nn * 512:(nn + 1) * 512],
                                    ckvT_f[:, ts_ * 4 + 2 * kc: ts_ * 4 + 2 * kc + 2,
                                           s_off:s_off + 128],
                                    v_eff_sb[:, 2 * kc:2 * kc + 2,
                                             nn * 512:(nn + 1) * 512],
                                    start=(kc == 0), stop=(kc == 1),
                                    perf_mode=DR)
                            else:
                                nc.tensor.matmul(
                                    pv[:, nn * 512:(nn + 1) * 512],
                                    ckvT_f[:, scb + kc * 512: scb + kc * 512 + 128],
                                    v_eff_sb[:, kc * D2 + nn * 512: kc * D2 + (nn + 1) * 512],
                                    start=(kc == 0), stop=(kc == 3))
                    if sc % 2:
                        nc.vector.tensor_copy(v_s[:, sc * D2:(sc + 1) * D2],
                                              pv[:])
                    else:
                        nc.scalar.activation(v_s[:, sc * D2:(sc + 1) * D2],
                                             pv[:], Copy)

                for hh in range(hpc_eff):
                    # roped q_r for a PAIR of heads at once (tile rows:
                    # head-even 0:64, head-odd 64:128), split on evac.
                    # Issued FIRST so the PE->DVE->Pool rope chain finishes
                    # under the kh/qh matmuls (logits j=0 needs qr).
                    if hh % 2 == 0:
                        pp = hh // 2
                        qrA = pqr.tile([DHR, T], bf16, tag="qrA")
                        qrB = pqr.tile([DHR, T], bf16, tag="qrB")
                        qrtmp = pqr.tile([128, 2 * 512], bf16, tag="qrtmp")
                        pr2 = [ps_work.tile([128, T], f32, tag="work",
                                            name=f"pr2_{hh}_{ts}")
                               for ts in range(2)]
                        # weight-chunk outer, ts inner: each stationary
                        # chunk serves both t-halves
                        for half in range(2):
                            for qc in range(2 if FP8Q else 4):
                                for ts in range(2):
                                    if FP8Q:
                                        nc.tensor.matmul(
                                            pr2[ts][:, half * 512:(half + 1) * 512],
                                            wqr2T_sb[:, pp * 8 + half * 4 + 2 * qc: pp * 8 + half * 4 + 2 * qc + 2, :],
                                            cqT_f[:, ts * 4 + 2 * qc: ts * 4 + 2 * qc + 2, :],
                                            start=(qc == 0), stop=(qc == 1),
                                            perf_mode=DR)
                                    else:
                                        nc.tensor.matmul(
                                            pr2[ts][:, half * 512:(half + 1) * 512],
                                            wqr2T_sb[:, pp * 1024 + half * 512 + qc * 128: pp * 1024 + half * 512 + (qc + 1) * 128],
                                            cqT_f[:, ts * 2048 + qc * 512: ts * 2048 + (qc + 1) * 512],
                                            start=(qc == 0), stop=(qc == 3))
                        for ts in range(2):
                            tsl = slice(ts * 512, (ts + 1) * 512)
                            nc.vector.tensor_tensor(
                                qrtmp[:, :512], pr2[ts][:, :512],
                                cos2d_sb[:, tsl], mult)
                            nc.vector.tensor_tensor(
                                qrtmp[:, 512:], pr2[ts][:, 512:],
                                sin2d_sb[:, tsl], mult)
                            nc.gpsimd.tensor_tensor(
                                qrA[:, tsl], qrtmp[:DHR, :512],
                                qrtmp[:DHR, 512:], add)
                            nc.gpsimd.tensor_tensor(
                                qrB[:, tsl], qrtmp[DHR:, :512],
                                qrtmp[DHR:, 512:], add)
                    qr = qrA if hh % 2 == 0 else qrB

                    # q_hT[d, t], k_hT[d, s] for the full batch; qc outer /
                    # ts inner reuses each stationary weight chunk twice;
                    # evacs split across DVE/ACT so neither serializes the
                    # next projection group
                    qh = ph.tile([128, T], bf16, tag="qh")
                    kh = ph.tile([128, T], bf16, tag="kh")
                    for ts in range(2):
                        pk = ps_proj.tile([128, 512], f32, tag="proj",
                                          name=f"pk_{hh}_{ts}")
                        for kc in range(2 if FP8KV else 4):
                            if FP8KV:
                                nc.tensor.matmul(
                                    pk[:],
                                    wukT_sb[:, hh * 4 + 2 * kc: hh * 4 + 2 * kc + 2, :],
                                    ckvT_f[:, ts * 4 + 2 * kc: ts * 4 + 2 * kc + 2, :],
                                    start=(kc == 0), stop=(kc == 1),
                                    perf_mode=DR)
                            else:
                                nc.tensor.matmul(
                                    pk[:],
                                    wukT_sb[:, hh * 512 + kc * 128: hh * 512 + (kc + 1) * 128],
                                    ckvT_f[:, ts * 2048 + kc * 512: ts * 2048 + (kc + 1) * 512],
                                    start=(kc == 0), stop=(kc == 3))
                        nc.vector.tensor_copy(kh[:, ts * 512:(ts + 1) * 512],
                                              pk[:])
                    for ts in range(2):
                        pq = ps_proj.tile([128, 512], f32, tag="proj",
                                          name=f"pq_{hh}_{ts}")
                        for qc in range(2 if FP8Q else 4):
                            if FP8Q:
                                nc.tensor.matmul(
                                    pq[:],
                                    wuqT_sb[:, hh * 4 + 2 * qc: hh * 4 + 2 * qc + 2, :],
                                    cqT_f[:, ts * 4 + 2 * qc: ts * 4 + 2 * qc + 2, :],
                                    start=(qc == 0), stop=(qc == 1),
                                    perf_mode=DR)
                            else:
                                nc.tensor.matmul(
                                    pq[:],
                                    wuqT_sb[:, hh * 512 + qc * 128: hh * 512 + (qc + 1) * 128],
                                    cqT_f[:, ts * 2048 + qc * 512: ts * 2048 + (qc + 1) * 512],
                                    start=(qc == 0), stop=(qc == 3))
                        nc.vector.tensor_copy(qh[:, ts * 512:(ts + 1) * 512],
                                              pq[:])

                    y_sb = py.tile([128, T], bf16, tag="y")
                    if oldsums:
                        sums_sb = py.tile([1, T], f32, tag="sums")
                    # pass 1: logits + exp, 8 s-chunks over the full t-range
                    # (one [128, njt] exp per chunk; lg spans 2 PSUM banks).
                    # content pair first then rope pair so each stationary
                    # (kh / krT chunk) is loaded once.
                    exs = []
                    for j in range(8):
                        t_off = 128 * j
                        lg = ps_work.tile([128, T], f32, tag="work")
                        if "wide" in variant:
                            csls = [slice(t_off, T)]
                        else:
                            csls = []
                            for th in range(2):
                                c0 = max(t_off, th * 512)
                                if c0 >= (th + 1) * 512:
                                    continue
                                csls.append(slice(c0, (th + 1) * 512))
                        for csl in csls:
                            nc.tensor.matmul(
                                lg[:, csl], kh[:, j * 128:(j + 1) * 128],
                                qh[:, csl], start=True, stop=False)
                        for csl in csls:
                            nc.tensor.matmul(
                                lg[:, csl], krT_f[:, j * 128:(j + 1) * 128],
                                qr[:, csl], start=False, stop=True)
                        ex = pex.tile([128, T], bf16, tag="ex",
                                      name=f"ex_{hh}_{j}")
                        nc.scalar.activation(ex[:, t_off:], lg[:, t_off:],
                                             Exp, scale=SCALE)
                        nc.gpsimd.tensor_tensor(
                            ex[:, t_off:t_off + 128], ex[:, t_off:t_off + 128],
                            mask_r[:], mult)
                        exs.append(ex)
                    # pass 2: AV + sums accumulation per t-half bank
                    for th in range(2):
                        av_ps = ps_av.tile([128, 512], f32, tag="av")
                        if "nosums" not in variant:
                            sums_ps = ps_sums.tile([1, 512], f32, tag="sums",
                                                   name=f"sums_{hh}_{th}")
                        njs = [j for j in range(8) if 128 * j < (th + 1) * 512]
                        for i, j in enumerate(njs):
                            c0 = max(128 * j, th * 512)
                            esl = slice(c0, (th + 1) * 512)
                            osl = slice(c0 - th * 512, 512)
                            first, last = (i == 0), (i == len(njs) - 1)
                            nc.tensor.matmul(
                                av_ps[:, osl],
                                v_s[:, j * D2 + hh * HS: j * D2 + (hh + 1) * HS],
                                exs[j][:, esl], start=first, stop=last)
                        for i, j in enumerate(njs):
                            if "nosums" in variant:
                                break
                            c0 = max(128 * j, th * 512)
                            esl = slice(c0, (th + 1) * 512)
                            osl = slice(c0 - th * 512, 512)
                            first, last = (i == 0), (i == len(njs) - 1)
                            nc.tensor.matmul(
                                sums_ps[:, osl], ones_r[:],
                                exs[j][:, esl], start=first, stop=last)
                        tsl = slice(th * 512, (th + 1) * 512)
                        if th == 0:
                            nc.scalar.activation(y_sb[:, tsl], av_ps[:], Copy)
                        else:
                            nc.vector.tensor_copy(y_sb[:, tsl], av_ps[:])
                        if "nosums" in variant:
                            pass
                        elif oldsums:
                            nc.scalar.activation(sums_sb[:, tsl], sums_ps[:],
                                                 Copy)
                        else:
                            r = (2 * hh + th) * 512
                            nc.vector.tensor_copy(
                                sums_sb[:, r:r + 512], sums_ps[:])
                    nc.sync.dma_start(out[hh * HS:(hh + 1) * HS, :], y_sb[:])
                    if oldsums:
                        nc.sync.dma_start(
                            out2[:, 2 * hh * 512:(2 * hh + 1) * 512],
                            sums_sb[:1, :512])
                        nc.sync.dma_start(
                            out2[:, (2 * hh + 1) * 512:(2 * hh + 2) * 512],
                            sums_sb[:1, 512:])
                if hpc_eff and not oldsums and "nosums" not in variant:
                    nc.sync.dma_start(out2[:, :], sums_sb[:])
    nc.compile()
    return nc


def _pairswap(w):
    idx = np.arange(w.shape[0]).reshape(-1, 2)[:, ::-1].reshape(-1)
    return w[idx]


def _slab(m, dtype):
    """[n*128, W] row-major -> SBUF slab layout [128, n*W]."""
    n = m.shape[0] // 128
    return np.ascontiguousarray(
        m.reshape(n, 128, m.shape[1]).transpose(1, 0, 2).reshape(128, -1),
        dtype=dtype)


def _make_in_maps(x, W_dq, W_uq, W_dkv, W_uk, W_uv, W_o, W_qr, W_kr,
                  freqs_cos, freqs_sin):
    import ml_dtypes
    f4 = np.float32
    bf = ml_dtypes.bfloat16
    kv = ml_dtypes.float8_e4m3 if FP8KV else bf
    qd = ml_dtypes.float8_e4m3 if FP8Q else bf
    wdqT = _slab(W_dq.T, bf)
    wdkvT = _slab(W_dkv.T, bf)
    wkr2T = _slab(np.concatenate([W_kr.T, _pairswap(W_kr).T], axis=1), bf)
    veff_full = W_uv.T.astype(f4) @ W_o.T.astype(f4)   # (NLKV, C) on host
    uq = W_uq.reshape(NLQ, NH, HS)
    uk = W_uk.reshape(NH, HS, NLKV)
    cos2 = np.repeat(freqs_cos.T, 2, axis=0).astype(f4)          # [DHR, T]
    sin_half = freqs_sin.T.astype(f4)                            # [DHR/2, T]
    sin2 = np.empty((DHR, T), dtype=f4)
    sin2[0::2] = -sin_half
    sin2[1::2] = sin_half
    cos2d = np.concatenate([cos2, cos2], axis=0)                 # [128, T]
    sin2d = np.concatenate([sin2, sin2], axis=0)

    in_maps = []
    for i in range(NCORES):
        b_own, half = divmod(i, 2)
        t0 = half * TOK
        heads = [HPC * half + hh for hh in range(HPC)]
        # per-head lhsT slabs: block hh at cols [hh*512 + qc*128]
        wuqT = np.concatenate(
            [_slab(uq[:, h, :], qd) for h in heads], axis=1)     # A_h (NLQ, HS)
        wukT = np.concatenate(
            [_slab(uk[h].T, kv) for h in heads], axis=1)         # B_h.T (NLKV, HS)
        qr_tiles = []
        for p in range(HPC // 2):
            hA, hB = heads[2 * p], heads[2 * p + 1]
            t1 = np.concatenate([W_qr[hA * DHR:(hA + 1) * DHR].T,
                                 W_qr[hB * DHR:(hB + 1) * DHR].T], axis=1)
            t2 = np.concatenate([_pairswap(W_qr[hA * DHR:(hA + 1) * DHR]).T,
                                 _pairswap(W_qr[hB * DHR:(hB + 1) * DHR]).T],
                                axis=1)
            qr_tiles += [t1, t2]
        wqr2T = np.concatenate([_slab(t, qd) for t in qr_tiles], axis=1)
        veffp = _slab(veff_full[:, heads[0] * HS:(heads[-1] + 1) * HS], kv)
        in_maps.append({
            "xT": _slab(x[b_own, t0:t0 + TOK, :].T, bf),
            "wdqT": wdqT, "wdkvT": wdkvT, "wkr2T": wkr2T,
            "wuqT": wuqT, "wukT": wukT, "wqr2T": wqr2T,
            "veffp": veffp,
            "cos2d": cos2d, "sin2d": sin2d,
            "cos2o": np.ascontiguousarray(cos2[:, t0:t0 + TOK]),
            "sin2o": np.ascontiguousarray(sin2[:, t0:t0 + TOK]),
            "maskp": np.triu(np.ones((128, 128))).astype(bf),
        })
    return in_maps


def _assemble(results):
    y = np.empty((B, T, C), dtype=np.float32)
    for i in range(NCORES):
        b_own, half = divmod(i, 2)
        o = np.asarray(results[i]["out"], dtype=np.float32)  # unnormalized
        s2 = results[i]["out2"].reshape(16, 512)  # row 2*hh+th = denoms
        for hh in range(HPC):
            h = HPC * half + hh
            den = np.concatenate([s2[2 * hh], s2[2 * hh + 1]])   # [T]
            blk = o[hh * HS:(hh + 1) * HS, :] / den
            y[b_own, :, h * HS:(h + 1) * HS] = blk.T
    return y


def kernel(**inputs):
    from concourse import bass_utils
    if "nc" not in _cache:
        _cache["nc"] = _build()
    nc = _cache["nc"]
    in_maps = _make_in_maps(**{k: np.asarray(v) for k, v in inputs.items()})
    res = bass_utils.run_bass_kernel_spmd(nc, in_maps, core_ids=list(range(NCORES)))
    return _assemble(res.results)


# revision 31
# speedup vs baseline: 1.0306x; 1.0306x over previous
"""MLA attention (DeepSeek-style) on 8 TRN2 NeuronCores — unabsorbed 2D
(batch-pair x head) sharding.

Sharding: cores (2b, 2b+1) own batch b. Core 2b runs heads 0-7, core 2b+1
heads 8-15, each over the full 1024 tokens of its batch. Preprocessing
(c_q, c_kv, roped k_r latents) is token-split within the pair (512 tokens
each) and exchanged with TWO pair-group AllGathers: c_kv right after it is
computed (so the exchange overlaps the c_q/k_r matmuls) and c_q+k_r after
phase 0 (overlapping the v_s/k_h matmuls at the start of attention).

Math: instead of the absorbed per-head k_eff = U_q @ U_k (making attention
contract over the 512-dim latent), we factor back to per-head q_h = c_q@U_q,
k_h = U_k@c_kv, v_h = c_kv@v_eff_h (HS=128-dim attention contraction) —
~2x fewer PE MACs, and AV output lands directly in the head's output block
(v_eff_h = (W_uv.T @ W_o.T) head-block keeps the output per-head local, so
no all-reduce).

Precision: bf16 everywhere on the PE (fp8-e4m3 DoubleRow was tried via the
FP8KV/FP8Q flags and REJECTED: rel-err 5.7e-2 > 2e-2 budget AND no speedup
— DoubleRow disables FWL so the halved column count is eaten by slower
weight loads); PSUM accumulation fp32. Attention is computed in the
transposed orientation logitsT[s, t] (no transposes anywhere; softmax
needs no max subtraction since logits are O(1)); column sums via an
appended ones-row matmul; the unnormalized y is written bf16; host divides
by the softmax denominators ([1, 8192] f32, one DMA).

Scheduling notes: DMA queue order is arranged so phase-0 inputs land
before the (large) resident attention weights (kills a ~16 us DMA lead-in
before the first matmul); q_r is issued before k_h/q_h on even heads so
the PE->DVE->Pool rope chain finishes under them; PSUM evacuations are
spread across ACT (exp, y th0, half the v_s/ckv) and DVE (projections,
y th1, sums) with masks on Pool, so the PE rarely waits on one engine.
Matmul N is capped at 512 (one PSUM bank) by ISA; per-MM overhead on HW
is ~125 ns (LDWEIGHTS largely unpipelined — walrus runs ldw-opt off), so
fewer-bigger matmuls and column-count cuts are the main levers. Measured:
removing the 96 sums matmuls (36864 cols) saves 16.7 us -> attention time
is ~proportional to PE columns at ~0.45 ns/col marginal.
"""

import math

import numpy as np

B, T, C = 4, 1024, 2048
NH, HS = 16, 128
NLQ = NLKV = 512
DHR = 64
NCORES = 8
HPC = 8                     # heads per core (half the heads, own batch)
TOK = 512                   # tokens per core in preprocessing (half a batch)
SCALE = 1.0 / math.sqrt(HS + DHR)
CC = C // 128               # 16 contraction chunks over C
FP8KV = False               # c_kv latent + U_k/v_eff in fp8-e4m3, DoubleRow
FP8Q = False                # c_q latent + U_q/W_qr in fp8-e4m3, DoubleRow

_cache = {}


def _build(loop_k=None, loop_pre=None, sim_single=False, phases="all",
           variant=""):
    """Build the SPMD kernel. loop_k / loop_pre: wrap the attention / the
    phase-0 body in a For_i hardware loop (timing amplification only).
    sim_single: single-core no-collective variant (gather outputs fed as
    inputs) for CoreSim/TimelineSim analysis. phases="pre" drops phase 2
    (cost-model phase attribution only)."""
    import contextlib

    import concourse.mybir as mybir
    import concourse.tile as tile
    from concourse import bacc

    f32 = mybir.dt.float32
    bf16 = mybir.dt.bfloat16
    fp8 = mybir.dt.float8e4
    kvdt = fp8 if FP8KV else bf16
    DR = mybir.MatmulPerfMode.DoubleRow
    Exp = mybir.ActivationFunctionType.Exp
    Copy = mybir.ActivationFunctionType.Copy
    mult = mybir.AluOpType.mult
    add = mybir.AluOpType.add

    nc = bacc.Bacc(trn_type="TRN2", num_devices=1 if sim_single else NCORES)
    P = nc.declare_dram_parameter

    xT = P("xT", [128, CC * TOK], bf16, isOutput=False)
    wdqT = P("wdqT", [128, CC * NLQ], bf16, isOutput=False)
    wdkvT = P("wdkvT", [128, CC * NLKV], bf16, isOutput=False)
    wkr2T = P("wkr2T", [128, CC * 2 * DHR], bf16, isOutput=False)
    wuqT = P("wuqT", [128, HPC * 512], fp8 if FP8Q else bf16, isOutput=False)
    wukT = P("wukT", [128, HPC * 512], kvdt, isOutput=False)
    wqr2T = P("wqr2T", [128, HPC * 1024 // 2], fp8 if FP8Q else bf16, isOutput=False)
    veffp = P("veffp", [128, 4 * HPC * HS], kvdt, isOutput=False)
    cos2d = P("cos2d", [128, T], f32, isOutput=False)
    sin2d = P("sin2d", [128, T], f32, isOutput=False)
    cos2o = P("cos2o", [DHR, TOK], f32, isOutput=False)
    sin2o = P("sin2o", [DHR, TOK], f32, isOutput=False)
    maskp = P("maskp", [128, 128], bf16, isOutput=False)
    out = P("out", [HPC * HS, T], bf16, isOutput=True)
    out2 = P("out2", [1, 16 * 512], f32, isOutput=True)
    ag1_p = ag2_p = ag3_p = None
    if sim_single:
        ag1_p = P("ag1_p", [2 * 128, 4 * TOK], kvdt, isOutput=False)
        ag2_p = P("ag2_p", [2 * 128, (4 if FP8Q else 5) * TOK],
                  fp8 if FP8Q else bf16, isOutput=False)
        if FP8Q:
            ag3_p = P("ag3_p", [2 * 128, TOK], bf16, isOutput=False)

    oldsums = "oldsums" in variant

    with tile.TileContext(nc) as tc:
        with (
            tc.tile_pool(name="pres", bufs=1) as pres,
            tc.tile_pool(name="dram", bufs=1, space="DRAM") as dram,
            # 8 PSUM banks total: work 2x[128,1024]=4, proj 2x[128,512]=2,
            # av 1, sums 1
            tc.tile_pool(name="ps_work", bufs=2, space="PSUM") as ps_work,
            tc.tile_pool(name="ps_proj", bufs=2, space="PSUM") as ps_proj,
            tc.tile_pool(name="ps_av", bufs=1, space="PSUM") as ps_av,
            tc.tile_pool(name="ps_sums", bufs=1, space="PSUM") as ps_sums,
        ):
            # ---------- resident tensors ----------
            wuqT_sb = (pres.tile([128, HPC * 4, 128], fp8, tag="wuqT",
                                 name="wuqT_sb")
                       if FP8Q else
                       pres.tile([128, HPC * 512], bf16, tag="wuqT",
                                 name="wuqT_sb"))
            wukT_sb = (pres.tile([128, HPC * 4, 128], fp8, tag="wukT", name="wukT_sb")
                       if FP8KV else
                       pres.tile([128, HPC * 512], bf16, tag="wukT", name="wukT_sb"))
            wqr2T_sb = (pres.tile([128, HPC * 4, 128], fp8, tag="wqr2T",
                                  name="wqr2T_sb")
                        if FP8Q else
                        pres.tile([128, HPC * 512], bf16, tag="wqr2T",
                                  name="wqr2T_sb"))
            cos2d_sb = pres.tile([128, T], f32, tag="cos2d")
            sin2d_sb = pres.tile([128, T], f32, tag="sin2d")
            cos2o_sb = pres.tile([DHR, TOK], f32, tag="cos2o")
            sin2o_sb = pres.tile([DHR, TOK], f32, tag="sin2o")
            v_eff_sb = (pres.tile([128, 4, HPC * HS], fp8, tag="v_eff", name="v_eff_sb")
                        if FP8KV else
                        pres.tile([128, 4 * HPC * HS], bf16, tag="v_eff", name="v_eff_sb"))
            cqT_f = (pres.tile([128, 8, 512], fp8, tag="cqT_f",
                               name="cqT_f")
                     if FP8Q else
                     pres.tile([128, 4 * T], bf16, tag="cqT_f",
                               name="cqT_f"))
            ckvT_f = (pres.tile([128, 8, 512], fp8, tag="ckvT_f", name="ckvT_f")
                      if FP8KV else
                      pres.tile([128, 4 * T], bf16, tag="ckvT_f", name="ckvT_f"))
            krT_f = pres.tile([DHR, T], bf16, tag="krT_f")
            ones_sb = pres.tile([128, 1], f32, tag="ones")
            ones_r = pres.tile([128, 1], bf16, tag="ones_r")
            mask_r = pres.tile([128, 128], bf16, tag="mask_r")

            nc.gpsimd.memset(ones_sb[:], 1.0)
            nc.vector.tensor_copy(ones_r[:], ones_sb[:])

            # DRAM bounce buffers for the pair AllGathers (Local output:
            # shared-output collectives need >4-core groups)
            agin1 = dram.tile([128, 4 * TOK], kvdt)    # c_kv
            agin2 = dram.tile([128, 5 * TOK], bf16)    # c_q ++ k_r
            agout1 = ag1_p if sim_single else dram.tile([2 * 128, 4 * TOK], kvdt)
            agout2 = ag2_p if sim_single else dram.tile([2 * 128, 5 * TOK], bf16)

            # ---------- phase 0: local latents (c_kv first) ----------
            # In loop-amplified builds the body is unrolled 2x with a
            # double-buffered xT so iteration i+1's input DMA overlaps
            # iteration i's compute (matches the one-shot kernel, where the
            # phase-0 loads have no prior compute to hide behind but also
            # run only once).
            def phase0(p0, u=0):
                xT_sb = p0.tile([128, CC * TOK], bf16, tag=f"xT{u}",
                                name=f"xT_sb{u}")
                wdqT_sb = p0.tile([128, CC * NLQ], bf16, tag="wdqT",
                                  name=f"wdqT_sb{u}")
                wdkvT_sb = p0.tile([128, CC * NLKV], bf16, tag="wdkvT",
                                   name=f"wdkvT_sb{u}")
                wkr2T_sb = p0.tile([128, CC * 2 * DHR], bf16, tag="wkr2T",
                                   name=f"wkr2T_sb{u}")
                cq_loc = p0.tile([128, 4 * TOK], fp8 if FP8Q else bf16,
                                 tag="cq_loc", name=f"cq_loc{u}")
                ckv_loc = p0.tile([128, 4 * TOK], kvdt, tag="ckv_loc",
                                  name=f"ckv_loc{u}")
                kr_loc = p0.tile([DHR, TOK], bf16, tag="kr_loc",
                                 name=f"kr_loc{u}")
                rtmp = p0.tile([DHR, 2 * TOK], f32, tag="rtmp",
                               name=f"rtmp{u}")

                # phase-0 inputs first: xT on SP ring, wdkvT on ACT ring so
                # the first c_kv matmul can start after one chunk of each.
                for qr_ in range(4):
                    csl = slice(qr_ * 4 * TOK, (qr_ + 1) * 4 * TOK)
                    nc.sync.dma_start(xT_sb[:, csl], xT[:, csl])
                    nc.scalar.dma_start(wdkvT_sb[:, csl], wdkvT[:, csl])
                for qr_ in range(4):
                    wsl = slice(qr_ * 4 * NLQ, (qr_ + 1) * 4 * NLQ)
                    nc.sync.dma_start(wdqT_sb[:, wsl], wdqT[:, wsl])
                nc.scalar.dma_start(cos2o_sb[:], cos2o[:])
                nc.scalar.dma_start(sin2o_sb[:], sin2o[:])
                nc.scalar.dma_start(wkr2T_sb[:], wkr2T[:, :])
                if "p0dma" in variant:
                    # DMA-rate probe: consume the tiles with one trivial op
                    # so the loop body depends on every load, no compute.
                    nc.vector.tensor_copy(kr_loc[:, :1], xT_sb[:DHR, :1])
                    nc.vector.tensor_copy(kr_loc[:, 1:2], wdqT_sb[:DHR, :1])
                    nc.vector.tensor_copy(kr_loc[:, 2:3], wdkvT_sb[:DHR, :1])
                    nc.vector.tensor_copy(kr_loc[:, 3:4], wkr2T_sb[:DHR, :1])
                    return

                # c_kv: 4 PSUM slots held across cc-groups so compute on DMA
                # chunk g overlaps the chunk g+1 loads
                pks = [(ps_proj if kt % 2 else ps_work).tile(
                    [128, TOK], f32, tag="proj" if kt % 2 else "work",
                    name=f"p0ckv_{kt}_{u}")
                    for kt in range(4)]
                for g in range(4):
                    for kt in range(4):
                        for cc in range(4 * g, 4 * g + 4):
                            nc.tensor.matmul(
                                pks[kt][:],
                                wdkvT_sb[:, cc * NLKV + kt * 128: cc * NLKV + (kt + 1) * 128],
                                xT_sb[:, cc * TOK:(cc + 1) * TOK],
                                start=(cc == 0), stop=(cc == CC - 1))
                for kt in range(4):
                    eng = nc.vector if kt % 2 else nc.scalar
                    if kt % 2:
                        eng.tensor_copy(ckv_loc[:, kt * TOK:(kt + 1) * TOK],
                                        pks[kt][:])
                    else:
                        eng.activation(ckv_loc[:, kt * TOK:(kt + 1) * TOK],
                                       pks[kt][:], Copy)
                nc.gpsimd.dma_start(agin1[:], ckv_loc[:])

                # c_q
                pqs = [(ps_proj if qt % 2 else ps_work).tile(
                    [128, TOK], f32, tag="proj" if qt % 2 else "work",
                    name=f"p0cq_{qt}_{u}")
                    for qt in range(4)]
                for g in range(4):
                    for qt in range(4):
                        for cc in range(4 * g, 4 * g + 4):
                            nc.tensor.matmul(
                                pqs[qt][:],
                                wdqT_sb[:, cc * NLQ + qt * 128: cc * NLQ + (qt + 1) * 128],
                                xT_sb[:, cc * TOK:(cc + 1) * TOK],
                                start=(cc == 0), stop=(cc == CC - 1))
                for qt in range(4):
                    eng = nc.vector if qt % 2 else nc.scalar
                    if qt % 2:
                        eng.tensor_copy(cq_loc[:, qt * TOK:(qt + 1) * TOK],
                                        pqs[qt][:])
                    else:
                        eng.activation(cq_loc[:, qt * TOK:(qt + 1) * TOK],
                                       pqs[qt][:], Copy)

                # roped k_r: rows 0..63 raw, 64..127 pair-swapped copy
                pr = ps_proj.tile([128, TOK], f32, tag="proj",
                                  name=f"p0kr_{u}")
                for cc in range(CC):
                    nc.tensor.matmul(
                        pr[:],
                        wkr2T_sb[:, cc * 2 * DHR:(cc + 1) * 2 * DHR],
                        xT_sb[:, cc * TOK:(cc + 1) * TOK],
                        start=(cc == 0), stop=(cc == CC - 1))
                nc.vector.tensor_tensor(rtmp[:, :TOK], pr[:DHR, :], cos2o_sb[:], mult)
                nc.vector.tensor_tensor(rtmp[:, TOK:], pr[DHR:, :], sin2o_sb[:], mult)
                nc.vector.tensor_tensor(kr_loc[:], rtmp[:, :TOK], rtmp[:, TOK:], add)

                if FP8Q:
                    nc.gpsimd.dma_start(agin2[:, :], cq_loc[:])
                    nc.gpsimd.dma_start(agin3[:DHR, :], kr_loc[:])
                    nc.gpsimd.dma_start(agin3[DHR:, :], kr_loc[:])
                else:
                    nc.gpsimd.dma_start(agin2[:, :4 * TOK], cq_loc[:])
                    nc.gpsimd.dma_start(agin2[:DHR, 4 * TOK:], kr_loc[:])
                    nc.gpsimd.dma_start(agin2[DHR:, 4 * TOK:], kr_loc[:])

            if phases != "attn":
                with (
                    tc.tile_pool(name="p0", bufs=1) as p0,
                    tc.For_i(0, loop_pre // 2, 1,
                             hint_engines=(mybir.EngineType.PE,))
                    if loop_pre else contextlib.nullcontext(),
                ):
                    phase0(p0, 0)
                    if loop_pre:
                        phase0(p0, 1)

            groups = [[2 * i, 2 * i + 1] for i in range(NCORES // 2)]
            if not sim_single and phases != "attn":
                nc.gpsimd.collective_compute(
                    "AllGather", mybir.AluOpType.bypass,
                    replica_groups=groups,
                    ins=[agin1.opt()], outs=[agout1.opt()])
                nc.gpsimd.collective_compute(
                    "AllGather", mybir.AluOpType.bypass,
                    replica_groups=groups,
                    ins=[agin2.opt()], outs=[agout2.opt()])
                if FP8Q:
                    nc.gpsimd.collective_compute(
                        "AllGather", mybir.AluOpType.bypass,
                        replica_groups=groups,
                        ins=[agin3.opt()], outs=[agout3.opt()])

            # resident attention weights, ordered by first use (after the
            # phase-0 inputs on both rings)
            nc.scalar.dma_start(v_eff_sb[:], veffp[:, :])
            nc.sync.dma_start(wukT_sb[:], wukT[:, :])
            nc.scalar.dma_start(wuqT_sb[:], wuqT[:, :])
            nc.sync.dma_start(wqr2T_sb[:], wqr2T[:, :])
            nc.sync.dma_start(cos2d_sb[:], cos2d[:])
            nc.scalar.dma_start(sin2d_sb[:], sin2d[:])
            nc.scalar.dma_start(mask_r[:], maskp[:])

            # ---------- phase 1: unpack gathered latents ----------
            # cqT_f/ckvT_f are r-major: col = ts*2048 + qc*512 + t_loc, so
            # each rank's block is ONE contiguous DMA from the gather output
            # split unpack across both HWDGE rings (SP + ACT) for parallelism
            ag1 = agout1.ap() if sim_single else agout1[:]
            ag2 = agout2.ap() if sim_single else agout2[:]
            ag3 = None
            if FP8Q:
                ag3 = agout3.ap() if sim_single else agout3[:]
            for r in range(2):
                rows = slice(r * 128, (r + 1) * 128)
                eng = nc.sync if r == 0 else nc.scalar
                if FP8KV:
                    eng.dma_start(ckvT_f[:, r * 4:(r + 1) * 4, :],
                                  ag1[rows, :])
                else:
                    eng.dma_start(
                        ckvT_f[:, r * 4 * TOK:(r + 1) * 4 * TOK],
                        ag1[rows, :])
                if FP8Q:
                    eng.dma_start(cqT_f[:, r * 4:(r + 1) * 4, :],
                                  ag2[rows, :])
                    eng.dma_start(
                        krT_f[:, r * TOK:(r + 1) * TOK],
                        ag3[r * 128: r * 128 + DHR, :])
                else:
                    eng.dma_start(
                        cqT_f[:, r * 4 * TOK:(r + 1) * 4 * TOK],
                        ag2[rows, :4 * TOK])
                    eng.dma_start(
                        krT_f[:, r * TOK:(r + 1) * TOK],
                        ag2[r * 128: r * 128 + DHR, 4 * TOK:])

            # ---------- phase 2: per-head projections + attention ----------
            with (
                tc.tile_pool(name="pv2", bufs=1) as pv2,
                tc.tile_pool(name="ph", bufs=2) as ph,
                tc.tile_pool(name="pqr", bufs=2) as pqr,
                tc.tile_pool(name="pex", bufs=14) as pex,
                tc.tile_pool(name="py", bufs=2) as py,
                tc.For_i(0, loop_k, 1, hint_engines=(mybir.EngineType.PE,))
                if loop_k else contextlib.nullcontext(),
            ):
                hpc_eff = 0 if phases == "pre" else HPC
                for tok in variant.split(","):
                    if tok.startswith("h") and tok[1:].isdigit():
                        hpc_eff = int(tok[1:])
                D2 = HPC * HS
                # softmax denominators: per head one PSUM bank with th=0 at
                # partition 0 and th=1 at partition 64 (both groups
                # accumulate concurrently); all 16 rows gathered in SBUF and
                # written out once
                if hpc_eff and not oldsums:
                    sums_sb = pv2.tile([1, 16 * 512], f32, tag="sums_sb")
                # v_s[s-chunk, d] for all 8 heads: [128, 8sc x (8h x 128d)]
                if hpc_eff:
                    v_s = pv2.tile([128, 8 * HPC * HS], bf16, tag="v_s")
                for sc in range(8 if hpc_eff else 0):
                    scb = (sc // 4) * 2048 + (sc % 4) * 128
                    pv = ps_work.tile([128, T], f32, tag="work")
                    # kc outer so the stationary ckv chunk is loaded once
                    # per (sc, kc) and reused by both nn matmuls
                    ts_, s_off = sc // 4, (sc % 4) * 128
                    for kc in range(2 if FP8KV else 4):
                        for nn in range(2):
                            if FP8KV:
                                nc.tensor.matmul(
                                    pv[:, nn * 512:(nn + 1) * 512],
                                    ckvT_f[:, ts_ * 4 + 2 * kc: ts_ * 4 + 2 * kc + 2,
                                           s_off:s_off + 128],
                                    v_eff_sb[:, 2 * kc:2 * kc + 2,
                                             nn * 512:(nn + 1) * 512],
                                    start=(kc == 0), stop=(kc == 1),
                                    perf_mode=DR)
                            else:
                                nc.tensor.matmul(
                                    pv[:, nn * 512:(nn + 1) * 512],
                                    ckvT_f[:, scb + kc * 512: scb + kc * 512 + 128],
                                    v_eff_sb[:, kc * D2 + nn * 512: kc * D2 + (nn + 1) * 512],
                                    start=(kc == 0), stop=(kc == 3))
                    if sc % 2:
                        nc.vector.tensor_copy(v_s[:, sc * D2:(sc + 1) * D2],
                                              pv[:])
                    else:
                        nc.scalar.activation(v_s[:, sc * D2:(sc + 1) * D2],
                                             pv[:], Copy)

                for hh in range(hpc_eff):
                    # roped q_r for a PAIR of heads at once (tile rows:
                    # head-even 0:64, head-odd 64:128), split on evac.
                    # Issued FIRST so the PE->DVE->Pool rope chain finishes
                    # under the kh/qh matmuls (logits j=0 needs qr).
                    if hh % 2 == 0:
                        pp = hh // 2
                        qrA = pqr.tile([DHR, T], bf16, tag="qrA")
                        qrB = pqr.tile([DHR, T], bf16, tag="qrB")
                        qrtmp = pqr.tile([128, 2 * 512], bf16, tag="qrtmp")
                        pr2 = [ps_work.tile([128, T], f32, tag="work",
                                            name=f"pr2_{hh}_{ts}")
                               for ts in range(2)]
                        # weight-chunk outer, ts inner: each stationary
                        # chunk serves both t-halves
                        for half in range(2):
                            for qc in range(2 if FP8Q else 4):
                                for ts in range(2):
                                    if FP8Q:
                                        nc.tensor.matmul(
                                            pr2[ts][:, half * 512:(half + 1) * 512],
                                            wqr2T_sb[:, pp * 8 + half * 4 + 2 * qc: pp * 8 + half * 4 + 2 * qc + 2, :],
                                            cqT_f[:, ts * 4 + 2 * qc: ts * 4 + 2 * qc + 2, :],
                                            start=(qc == 0), stop=(qc == 1),
                                            perf_mode=DR)
                                    else:
                                        nc.tensor.matmul(
                                            pr2[ts][:, half * 512:(half + 1) * 512],
                                            wqr2T_sb[:, pp * 1024 + half * 512 + qc * 128: pp * 1024 + half * 512 + (qc + 1) * 128],
                                            cqT_f[:, ts * 2048 + qc * 512: ts * 2048 + (qc + 1) * 512],
                                            start=(qc == 0), stop=(qc == 3))
                        for ts in range(2):
                            tsl = slice(ts * 512, (ts + 1) * 512)
                            nc.vector.tensor_tensor(
                                qrtmp[:, :512], pr2[ts][:, :512],
                                cos2d_sb[:, tsl], mult)
                            nc.vector.tensor_tensor(
                                qrtmp[:, 512:], pr2[ts][:, 512:],
                                sin2d_sb[:, tsl], mult)
                            nc.gpsimd.tensor_tensor(
                                qrA[:, tsl], qrtmp[:DHR, :512],
                                qrtmp[:DHR, 512:], add)
                            nc.gpsimd.tensor_tensor(
                                qrB[:, tsl], qrtmp[DHR:, :512],
                                qrtmp[DHR:, 512:], add)
                    qr = qrA if hh % 2 == 0 else qrB

                    # q_hT[d, t], k_hT[d, s] for the full batch; qc outer /
                    # ts inner reuses each stationary weight chunk twice;
                    # evacs split across DVE/ACT so neither serializes the
                    # next projection group
                    qh = ph.tile([128, T], bf16, tag="qh")
                    kh = ph.tile([128, T], bf16, tag="kh")
                    for ts in range(2):
                        pk = ps_proj.tile([128, 512], f32, tag="proj",
                                          name=f"pk_{hh}_{ts}")
                        for kc in range(2 if FP8KV else 4):
                            if FP8KV:
                                nc.tensor.matmul(
                                    pk[:],
                                    wukT_sb[:, hh * 4 + 2 * kc: hh * 4 + 2 * kc + 2, :],
                                    ckvT_f[:, ts * 4 + 2 * kc: ts * 4 + 2 * kc + 2, :],
                                    start=(kc == 0), stop=(kc == 1),
                                    perf_mode=DR)
                            else:
                                nc.tensor.matmul(
                                    pk[:],
                                    wukT_sb[:, hh * 512 + kc * 128: hh * 512 + (kc + 1) * 128],
                                    ckvT_f[:, ts * 2048 + kc * 512: ts * 2048 + (kc + 1) * 512],
                                    start=(kc == 0), stop=(kc == 3))
                        nc.vector.tensor_copy(kh[:, ts * 512:(ts + 1) * 512],
                                              pk[:])
                    for ts in range(2):
                        pq = ps_proj.tile([128, 512], f32, tag="proj",
                                          name=f"pq_{hh}_{ts}")
                        for qc in range(2 if FP8Q else 4):
                            if FP8Q:
                                nc.tensor.matmul(
                                    pq[:],
                                    wuqT_sb[:, hh * 4 + 2 * qc: hh * 4 + 2 * qc + 2, :],
                                    cqT_f[:, ts * 4 + 2 * qc: ts * 4 + 2 * qc + 2, :],
                                    start=(qc == 0), stop=(qc == 1),
                                    perf_mode=DR)
                            else:
                                nc.tensor.matmul(
                                    pq[:],
                                    wuqT_sb[:, hh * 512 + qc * 128: hh * 512 + (qc + 1) * 128],
                                    cqT_f[:, ts * 2048 + qc * 512: ts * 2048 + (qc + 1) * 512],
                                    start=(qc == 0), stop=(qc == 3))
                        nc.vector.tensor_copy(qh[:, ts * 512:(ts + 1) * 512],
                                              pq[:])

                    y_sb = py.tile([128, T], bf16, tag="y")
                    if oldsums:
                        sums_sb = py.tile([1, T], f32, tag="sums")
                    # pass 1: logits + exp, 8 s-chunks over the full t-range
                    # (one [128, njt] exp per chunk; lg spans 2 PSUM banks).
                    # content pair first then rope pair so each stationary
                    # (kh / krT chunk) is loaded once.
                    exs = []
                    for j in range(8):
                        t_off = 128 * j
                        lg = ps_work.tile([128, T], f32, tag="work")
                        if "wide" in variant:
                            csls = [slice(t_off, T)]
                        else:
                            csls = []
                            for th in range(2):
                                c0 = max(t_off, th * 512)
                                if c0 >= (th + 1) * 512:
                                    continue
                                csls.append(slice(c0, (th + 1) * 512))
                        for csl in csls:
                            nc.tensor.matmul(
                                lg[:, csl], kh[:, j * 128:(j + 1) * 128],
                                qh[:, csl], start=True, stop=False)
                        for csl in csls:
                            nc.tensor.matmul(
                                lg[:, csl], krT_f[:, j * 128:(j + 1) * 128],
                                qr[:, csl], start=False, stop=True)
                        ex = pex.tile([128, T], bf16, tag="ex",
                                      name=f"ex_{hh}_{j}")
                        nc.scalar.activation(ex[:, t_off:], lg[:, t_off:],
                                             Exp, scale=SCALE)
                        nc.gpsimd.tensor_tensor(
                            ex[:, t_off:t_off + 128], ex[:, t_off:t_off + 128],
                            mask_r[:], mult)
                        exs.append(ex)
                    # pass 2: AV + sums accumulation per t-half bank
                    for th in range(2):
                        av_ps = ps_av.tile([128, 512], f32, tag="av")
                        if "nosums" not in variant:
                            sums_ps = ps_sums.tile([1, 512], f32, tag="sums",
                                                   name=f"sums_{hh}_{th}")
                        njs = [j for j in range(8) if 128 * j < (th + 1) * 512]
                        for i, j in enumerate(njs):
                            c0 = max(128 * j, th * 512)
                            esl = slice(c0, (th + 1) * 512)
                            osl = slice(c0 - th * 512, 512)
                            first, last = (i == 0), (i == len(njs) - 1)
                            nc.tensor.matmul(
                                av_ps[:, osl],
                                v_s[:, j * D2 + hh * HS: j * D2 + (hh + 1) * HS],
                                exs[j][:, esl], start=first, stop=last)
                        for i, j in enumerate(njs):
                            if "nosums" in variant:
                                break
                            c0 = max(128 * j, th * 512)
                            esl = slice(c0, (th + 1) * 512)
                            osl = slice(c0 - th * 512, 512)
                            first, last = (i == 0), (i == len(njs) - 1)
                            nc.tensor.matmul(
                                sums_ps[:, osl], ones_r[:],
                                exs[j][:, esl], start=first, stop=last)
                        tsl = slice(th * 512, (th + 1) * 512)
                        if th == 0:
                            nc.scalar.activation(y_sb[:, tsl], av_ps[:], Copy)
                        else:
                            nc.vector.tensor_copy(y_sb[:, tsl], av_ps[:])
                        if "nosums" in variant:
                            pass
                        elif oldsums:
                            nc.scalar.activation(sums_sb[:, tsl], sums_ps[:],
                                                 Copy)
                        else:
                            r = (2 * hh + th) * 512
                            nc.vector.tensor_copy(
                                sums_sb[:, r:r + 512], sums_ps[:])
                    nc.sync.dma_start(out[hh * HS:(hh + 1) * HS, :], y_sb[:])
                    if oldsums:
                        nc.sync.dma_start(
                            out2[:, 2 * hh * 512:(2 * hh + 1) * 512],
                            sums_sb[:1, :512])
                        nc.sync.dma_start(
                            out2[:, (2 * hh + 1) * 512:(2 * hh + 2) * 512],
                            sums_sb[:1, 512:])
                if hpc_eff and not oldsums and "nosums" not in variant:
                    nc.sync.dma_start(out2[:, :], sums_sb[:])
    nc.compile()
    return nc


def _pairswap(w):
    idx = np.arange(w.shape[0]).reshape(-1, 2)[:, ::-1].reshape(-1)
    return w[idx]


def _slab(m, dtype):
    """[n*128, W] row-major -> SBUF slab layout [128, n*W]."""
    n = m.shape[0] // 128
    return np.ascontiguousarray(
        m.reshape(n, 128, m.shape[1]).transpose(1, 0, 2).reshape(128, -1),
        dtype=dtype)


def _make_in_maps(x, W_dq, W_uq, W_dkv, W_uk, W_uv, W_o, W_qr, W_kr,
                  freqs_cos, freqs_sin):
    import ml_dtypes
    f4 = np.float32
    bf = ml_dtypes.bfloat16
    kv = ml_dtypes.float8_e4m3 if FP8KV else bf
    qd = ml_dtypes.float8_e4m3 if FP8Q else bf
    wdqT = _slab(W_dq.T, bf)
    wdkvT = _slab(W_dkv.T, bf)
    wkr2T = _slab(np.concatenate([W_kr.T, _pairswap(W_kr).T], axis=1), bf)
    veff_full = W_uv.T.astype(f4) @ W_o.T.astype(f4)   # (NLKV, C) on host
    uq = W_uq.reshape(NLQ, NH, HS)
    uk = W_uk.reshape(NH, HS, NLKV)
    cos2 = np.repeat(freqs_cos.T, 2, axis=0).astype(f4)          # [DHR, T]
    sin_half = freqs_sin.T.astype(f4)                            # [DHR/2, T]
    sin2 = np.empty((DHR, T), dtype=f4)
    sin2[0::2] = -sin_half
    sin2[1::2] = sin_half
    cos2d = np.concatenate([cos2, cos2], axis=0)                 # [128, T]
    sin2d = np.concatenate([sin2, sin2], axis=0)

    in_maps = []
    for i in range(NCORES):
        b_own, half = divmod(i, 2)
        t0 = half * TOK
        heads = [HPC * half + hh for hh in range(HPC)]
        # per-head lhsT slabs: block hh at cols [hh*512 + qc*128]
        wuqT = np.concatenate(
            [_slab(uq[:, h, :], qd) for h in heads], axis=1)     # A_h (NLQ, HS)
        wukT = np.concatenate(
            [_slab(uk[h].T, kv) for h in heads], axis=1)         # B_h.T (NLKV, HS)
        qr_tiles = []
        for p in range(HPC // 2):
            hA, hB = heads[2 * p], heads[2 * p + 1]
            t1 = np.concatenate([W_qr[hA * DHR:(hA + 1) * DHR].T,
                                 W_qr[hB * DHR:(hB + 1) * DHR].T], axis=1)
            t2 = np.concatenate([_pairswap(W_qr[hA * DHR:(hA + 1) * DHR]).T,
                                 _pairswap(W_qr[hB * DHR:(hB + 1) * DHR]).T],
                                axis=1)
            qr_tiles += [t1, t2]
        wqr2T = np.concatenate([_slab(t, qd) for t in qr_tiles], axis=1)
        veffp = _slab(veff_full[:, heads[0] * HS:(heads[-1] + 1) * HS], kv)
        in_maps.append({
            "xT": _slab(x[b_own, t0:t0 + TOK, :].T, bf),
            "wdqT": wdqT, "wdkvT": wdkvT, "wkr2T": wkr2T,
            "wuqT": wuqT, "wukT": wukT, "wqr2T": wqr2T,
            "veffp": veffp,
            "cos2d": cos2d, "sin2d": sin2d,
            "cos2o": np.ascontiguousarray(cos2[:, t0:t0 + TOK]),
            "sin2o": np.ascontiguousarray(sin2[:, t0:t0 + TOK]),
            "maskp": np.triu(np.ones((128, 128))).astype(bf),
        })
    return in_maps


def _assemble(results):
    y = np.empty((B, T, C), dtype=np.float32)
    for i in range(NCORES):
        b_own, half = divmod(i, 2)
        o = np.asarray(results[i]["out"], dtype=np.float32)  # unnormalized
        s2 = results[i]["out2"].reshape(16, 512)  # row 2*hh+th = denoms
        for hh in range(HPC):
            h = HPC * half + hh
            den = np.concatenate([s2[2 * hh], s2[2 * hh + 1]])   # [T]
            blk = o[hh * HS:(hh + 1) * HS, :] / den
            y[b_own, :, h * HS:(h + 1) * HS] = blk.T
    return y


def kernel(**inputs):
    from concourse import bass_utils
    if "nc" not in _cache:
        _cache["nc"] = _build()
    nc = _cache["nc"]
    in_maps = _make_in_maps(**{k: np.asarray(v) for k, v in inputs.items()})
    res = bass_utils.run_bass_kernel_spmd(nc, in_maps, core_ids=list(range(NCORES)))
    return _assemble(res.results)


# revision 33
# speedup vs baseline: 1.0370x; 1.0062x over previous
"""MLA attention (DeepSeek-style) on 8 TRN2 NeuronCores — unabsorbed 2D
(batch-pair x head) sharding.

Sharding: cores (2b, 2b+1) own batch b. Core 2b runs heads 0-7, core 2b+1
heads 8-15, each over the full 1024 tokens of its batch. Preprocessing
(c_q, c_kv, roped k_r latents) is token-split within the pair (512 tokens
each) and exchanged with TWO pair-group AllGathers: c_kv right after it is
computed (so the exchange overlaps the c_q/k_r matmuls) and c_q+k_r after
phase 0 (overlapping the v_s/k_h matmuls at the start of attention).

Math: instead of the absorbed per-head k_eff = U_q @ U_k (making attention
contract over the 512-dim latent), we factor back to per-head q_h = c_q@U_q,
k_h = U_k@c_kv, v_h = c_kv@v_eff_h (HS=128-dim attention contraction) —
~2x fewer PE MACs, and AV output lands directly in the head's output block
(v_eff_h = (W_uv.T @ W_o.T) head-block keeps the output per-head local, so
no all-reduce).

Precision: bf16 everywhere on the PE (fp8-e4m3 DoubleRow was tried via the
FP8KV/FP8Q flags and REJECTED: rel-err 5.7e-2 > 2e-2 budget AND no speedup
— DoubleRow disables FWL so the halved column count is eaten by slower
weight loads); PSUM accumulation fp32. Attention is computed in the
transposed orientation logitsT[s, t] (no transposes anywhere; softmax
needs no max subtraction since logits are O(1)); column sums via an
appended ones-row matmul; the unnormalized y is written bf16; host divides
by the softmax denominators ([1, 8192] f32, one DMA).

Scheduling notes: DMA queue order is arranged so phase-0 inputs land
before the (large) resident attention weights (kills a ~16 us DMA lead-in
before the first matmul); q_r is issued before k_h/q_h on even heads so
the PE->DVE->Pool rope chain finishes under them; PSUM evacuations are
spread across ACT (exp, y th0, half the v_s/ckv) and DVE (projections,
y th1, sums) with masks on Pool, so the PE rarely waits on one engine.
Matmul N is capped at 512 (one PSUM bank) by ISA; per-MM overhead on HW
is ~125 ns (LDWEIGHTS largely unpipelined — walrus runs ldw-opt off), so
fewer-bigger matmuls and column-count cuts are the main levers. Measured:
removing the 96 sums matmuls (36864 cols) saves 16.7 us -> attention time
is ~proportional to PE columns at ~0.45 ns/col marginal.
"""

import math

import numpy as np

B, T, C = 4, 1024, 2048
NH, HS = 16, 128
NLQ = NLKV = 512
DHR = 64
NCORES = 8
HPC = 8                     # heads per core (half the heads, own batch)
TOK = 512                   # tokens per core in preprocessing (half a batch)
SCALE = 1.0 / math.sqrt(HS + DHR)
CC = C // 128               # 16 contraction chunks over C
FP8KV = False               # c_kv latent + U_k/v_eff in fp8-e4m3, DoubleRow
FP8Q = False                # c_q latent + U_q/W_qr in fp8-e4m3, DoubleRow

_cache = {}


def _build(loop_k=None, loop_pre=None, sim_single=False, phases="all",
           variant=""):
    """Build the SPMD kernel. loop_k / loop_pre: wrap the attention / the
    phase-0 body in a For_i hardware loop (timing amplification only).
    sim_single: single-core no-collective variant (gather outputs fed as
    inputs) for CoreSim/TimelineSim analysis. phases="pre" drops phase 2
    (cost-model phase attribution only)."""
    import contextlib

    import concourse.mybir as mybir
    import concourse.tile as tile
    from concourse import bacc

    f32 = mybir.dt.float32
    bf16 = mybir.dt.bfloat16
    fp8 = mybir.dt.float8e4
    kvdt = fp8 if FP8KV else bf16
    DR = mybir.MatmulPerfMode.DoubleRow
    Exp = mybir.ActivationFunctionType.Exp
    Copy = mybir.ActivationFunctionType.Copy
    mult = mybir.AluOpType.mult
    add = mybir.AluOpType.add

    nc = bacc.Bacc(trn_type="TRN2", num_devices=1 if sim_single else NCORES)
    P = nc.declare_dram_parameter

    xT = P("xT", [128, CC * TOK], bf16, isOutput=False)
    wdqT = P("wdqT", [128, CC * NLQ], bf16, isOutput=False)
    wdkvT = P("wdkvT", [128, CC * NLKV], bf16, isOutput=False)
    wkr2T = P("wkr2T", [128, CC * 2 * DHR], bf16, isOutput=False)
    wuqT = P("wuqT", [128, HPC * 512], fp8 if FP8Q else bf16, isOutput=False)
    wukT = P("wukT", [128, HPC * 512], kvdt, isOutput=False)
    wqr2T = P("wqr2T", [128, HPC * 512 // 2], bf16, isOutput=False)
    veffp = P("veffp", [128, 4 * HPC * HS], kvdt, isOutput=False)
    cos2d = P("cos2d", [128, T], bf16, isOutput=False)
    sin2d = P("sin2d", [128, T], f32, isOutput=False)
    cos2o = P("cos2o", [DHR, TOK], f32, isOutput=False)
    sin2o = P("sin2o", [DHR, TOK], f32, isOutput=False)
    maskp = P("maskp", [128, 128], bf16, isOutput=False)
    permp = P("permp", [128, 128], bf16, isOutput=False)
    out = P("out", [HPC * HS, T], bf16, isOutput=True)
    out2 = P("out2", [1, 16 * 512], f32, isOutput=True)
    ag1_p = ag2_p = ag3_p = None
    if sim_single:
        ag1_p = P("ag1_p", [2 * 128, 4 * TOK], kvdt, isOutput=False)
        ag2_p = P("ag2_p", [2 * 128, (4 if FP8Q else 5) * TOK],
                  fp8 if FP8Q else bf16, isOutput=False)
        if FP8Q:
            ag3_p = P("ag3_p", [2 * 128, TOK], bf16, isOutput=False)

    oldsums = "oldsums" in variant

    with tile.TileContext(nc) as tc:
        with (
            tc.tile_pool(name="pres", bufs=1) as pres,
            tc.tile_pool(name="dram", bufs=1, space="DRAM") as dram,
            # 8 PSUM banks total: work 2x[128,1024]=4, proj 2x[128,512]=2,
            # av 1, sums 1
            tc.tile_pool(name="ps_work", bufs=2, space="PSUM") as ps_work,
            tc.tile_pool(name="ps_proj", bufs=2, space="PSUM") as ps_proj,
            tc.tile_pool(name="ps_av", bufs=1, space="PSUM") as ps_av,
            tc.tile_pool(name="ps_sums", bufs=1, space="PSUM") as ps_sums,
        ):
            # ---------- resident tensors ----------
            wuqT_sb = (pres.tile([128, HPC * 4, 128], fp8, tag="wuqT",
                                 name="wuqT_sb")
                       if FP8Q else
                       pres.tile([128, HPC * 512], bf16, tag="wuqT",
                                 name="wuqT_sb"))
            wukT_sb = (pres.tile([128, HPC * 4, 128], fp8, tag="wukT", name="wukT_sb")
                       if FP8KV else
                       pres.tile([128, HPC * 512], bf16, tag="wukT", name="wukT_sb"))
            wqr2T_sb = pres.tile([128, HPC * 256], bf16, tag="wqr2T",
                                 name="wqr2T_sb")
            cos2d_sb = pres.tile([128, T], bf16, tag="cos2d")
            sin2d_sb = pres.tile([128, T], f32, tag="sin2d")
            cos2o_sb = pres.tile([DHR, TOK], f32, tag="cos2o")
            sin2o_sb = pres.tile([DHR, TOK], f32, tag="sin2o")
            v_eff_sb = (pres.tile([128, 4, HPC * HS], fp8, tag="v_eff", name="v_eff_sb")
                        if FP8KV else
                        pres.tile([128, 4 * HPC * HS], bf16, tag="v_eff", name="v_eff_sb"))
            cqT_f = (pres.tile([128, 8, 512], fp8, tag="cqT_f",
                               name="cqT_f")
                     if FP8Q else
                     pres.tile([128, 4 * T], bf16, tag="cqT_f",
                               name="cqT_f"))
            ckvT_f = (pres.tile([128, 8, 512], fp8, tag="ckvT_f", name="ckvT_f")
                      if FP8KV else
                      pres.tile([128, 4 * T], bf16, tag="ckvT_f", name="ckvT_f"))
            krT_f = pres.tile([DHR, T], bf16, tag="krT_f")
            ones_sb = pres.tile([128, 1], f32, tag="ones")
            ones_r = pres.tile([128, 1], bf16, tag="ones_r")
            mask_r = pres.tile([128, 128], bf16, tag="mask_r")
            perm_sb = pres.tile([128, 128], bf16, tag="perm")

            nc.gpsimd.memset(ones_sb[:], 1.0)
            nc.vector.tensor_copy(ones_r[:], ones_sb[:])

            # DRAM bounce buffers for the pair AllGathers (Local output:
            # shared-output collectives need >4-core groups)
            agin1 = dram.tile([128, 4 * TOK], kvdt)    # c_kv
            agin2 = dram.tile([128, 5 * TOK], bf16)    # c_q ++ k_r
            agout1 = ag1_p if sim_single else dram.tile([2 * 128, 4 * TOK], kvdt)
            agout2 = ag2_p if sim_single else dram.tile([2 * 128, 5 * TOK], bf16)

            # ---------- phase 0: local latents (c_kv first) ----------
            # In loop-amplified builds the body is unrolled 2x with a
            # double-buffered xT so iteration i+1's input DMA overlaps
            # iteration i's compute (matches the one-shot kernel, where the
            # phase-0 loads have no prior compute to hide behind but also
            # run only once).
            def phase0(p0, u=0):
                xT_sb = p0.tile([128, CC * TOK], bf16, tag=f"xT{u}",
                                name=f"xT_sb{u}")
                wdqT_sb = p0.tile([128, CC * NLQ], bf16, tag="wdqT",
                                  name=f"wdqT_sb{u}")
                wdkvT_sb = p0.tile([128, CC * NLKV], bf16, tag="wdkvT",
                                   name=f"wdkvT_sb{u}")
                wkr2T_sb = p0.tile([128, CC * 2 * DHR], bf16, tag="wkr2T",
                                   name=f"wkr2T_sb{u}")
                cq_loc = p0.tile([128, 4 * TOK], fp8 if FP8Q else bf16,
                                 tag="cq_loc", name=f"cq_loc{u}")
                ckv_loc = p0.tile([128, 4 * TOK], kvdt, tag="ckv_loc",
                                  name=f"ckv_loc{u}")
                kr_loc = p0.tile([DHR, TOK], bf16, tag="kr_loc",
                                 name=f"kr_loc{u}")
                rtmp = p0.tile([DHR, 2 * TOK], f32, tag="rtmp",
                               name=f"rtmp{u}")

                # phase-0 inputs first: xT on SP ring, wdkvT on ACT ring so
                # the first c_kv matmul can start after one chunk of each.
                for qr_ in range(4):
                    csl = slice(qr_ * 4 * TOK, (qr_ + 1) * 4 * TOK)
                    nc.sync.dma_start(xT_sb[:, csl], xT[:, csl])
                    nc.scalar.dma_start(wdkvT_sb[:, csl], wdkvT[:, csl])
                for qr_ in range(4):
                    wsl = slice(qr_ * 4 * NLQ, (qr_ + 1) * 4 * NLQ)
                    nc.sync.dma_start(wdqT_sb[:, wsl], wdqT[:, wsl])
                nc.scalar.dma_start(cos2o_sb[:], cos2o[:])
                nc.scalar.dma_start(sin2o_sb[:], sin2o[:])
                nc.scalar.dma_start(wkr2T_sb[:], wkr2T[:, :])
                if "p0dma" in variant:
                    # DMA-rate probe: consume the tiles with one trivial op
                    # so the loop body depends on every load, no compute.
                    nc.vector.tensor_copy(kr_loc[:, :1], xT_sb[:DHR, :1])
                    nc.vector.tensor_copy(kr_loc[:, 1:2], wdqT_sb[:DHR, :1])
                    nc.vector.tensor_copy(kr_loc[:, 2:3], wdkvT_sb[:DHR, :1])
                    nc.vector.tensor_copy(kr_loc[:, 3:4], wkr2T_sb[:DHR, :1])
                    return

                # c_kv: 4 PSUM slots held across cc-groups so compute on DMA
                # chunk g overlaps the chunk g+1 loads
                pks = [(ps_proj if kt % 2 else ps_work).tile(
                    [128, TOK], f32, tag="proj" if kt % 2 else "work",
                    name=f"p0ckv_{kt}_{u}")
                    for kt in range(4)]
                for g in range(4):
                    for kt in range(4):
                        for cc in range(4 * g, 4 * g + 4):
                            nc.tensor.matmul(
                                pks[kt][:],
                                wdkvT_sb[:, cc * NLKV + kt * 128: cc * NLKV + (kt + 1) * 128],
                                xT_sb[:, cc * TOK:(cc + 1) * TOK],
                                start=(cc == 0), stop=(cc == CC - 1))
                for kt in range(4):
                    eng = nc.vector if kt % 2 else nc.scalar
                    if kt % 2:
                        eng.tensor_copy(ckv_loc[:, kt * TOK:(kt + 1) * TOK],
                                        pks[kt][:])
                    else:
                        eng.activation(ckv_loc[:, kt * TOK:(kt + 1) * TOK],
                                       pks[kt][:], Copy)
                nc.gpsimd.dma_start(agin1[:], ckv_loc[:])

                # c_q
                pqs = [(ps_proj if qt % 2 else ps_work).tile(
                    [128, TOK], f32, tag="proj" if qt % 2 else "work",
                    name=f"p0cq_{qt}_{u}")
                    for qt in range(4)]
                for g in range(4):
                    for qt in range(4):
                        for cc in range(4 * g, 4 * g + 4):
                            nc.tensor.matmul(
                                pqs[qt][:],
                                wdqT_sb[:, cc * NLQ + qt * 128: cc * NLQ + (qt + 1) * 128],
                                xT_sb[:, cc * TOK:(cc + 1) * TOK],
                                start=(cc == 0), stop=(cc == CC - 1))
                for qt in range(4):
                    eng = nc.vector if qt % 2 else nc.scalar
                    if qt % 2:
                        eng.tensor_copy(cq_loc[:, qt * TOK:(qt + 1) * TOK],
                                        pqs[qt][:])
                    else:
                        eng.activation(cq_loc[:, qt * TOK:(qt + 1) * TOK],
                                       pqs[qt][:], Copy)

                # roped k_r: rows 0..63 raw, 64..127 pair-swapped copy
                pr = ps_proj.tile([128, TOK], f32, tag="proj",
                                  name=f"p0kr_{u}")
                for cc in range(CC):
                    nc.tensor.matmul(
                        pr[:],
                        wkr2T_sb[:, cc * 2 * DHR:(cc + 1) * 2 * DHR],
                        xT_sb[:, cc * TOK:(cc + 1) * TOK],
                        start=(cc == 0), stop=(cc == CC - 1))
                nc.vector.tensor_tensor(rtmp[:, :TOK], pr[:DHR, :], cos2o_sb[:], mult)
                nc.vector.tensor_tensor(rtmp[:, TOK:], pr[DHR:, :], sin2o_sb[:], mult)
                nc.vector.tensor_tensor(kr_loc[:], rtmp[:, :TOK], rtmp[:, TOK:], add)

                if FP8Q:
                    nc.gpsimd.dma_start(agin2[:, :], cq_loc[:])
                    nc.gpsimd.dma_start(agin3[:DHR, :], kr_loc[:])
                    nc.gpsimd.dma_start(agin3[DHR:, :], kr_loc[:])
                else:
                    nc.gpsimd.dma_start(agin2[:, :4 * TOK], cq_loc[:])
                    nc.gpsimd.dma_start(agin2[:DHR, 4 * TOK:], kr_loc[:])
                    nc.gpsimd.dma_start(agin2[DHR:, 4 * TOK:], kr_loc[:])

            if phases != "attn":
                with (
                    tc.tile_pool(name="p0", bufs=1) as p0,
                    tc.For_i(0, loop_pre // 2, 1,
                             hint_engines=(mybir.EngineType.PE,))
                    if loop_pre else contextlib.nullcontext(),
                ):
                    phase0(p0, 0)
                    if loop_pre:
                        phase0(p0, 1)

            groups = [[2 * i, 2 * i + 1] for i in range(NCORES // 2)]
            if not sim_single and phases != "attn":
                nc.gpsimd.collective_compute(
                    "AllGather", mybir.AluOpType.bypass,
                    replica_groups=groups,
                    ins=[agin1.opt()], outs=[agout1.opt()])
                nc.gpsimd.collective_compute(
                    "AllGather", mybir.AluOpType.bypass,
                    replica_groups=groups,
                    ins=[agin2.opt()], outs=[agout2.opt()])
                if FP8Q:
                    nc.gpsimd.collective_compute(
                        "AllGather", mybir.AluOpType.bypass,
                        replica_groups=groups,
                        ins=[agin3.opt()], outs=[agout3.opt()])

            # resident attention weights, ordered by first use (after the
            # phase-0 inputs on both rings)
            nc.scalar.dma_start(v_eff_sb[:], veffp[:, :])
            nc.sync.dma_start(wukT_sb[:], wukT[:, :])
            nc.scalar.dma_start(wuqT_sb[:], wuqT[:, :])
            nc.sync.dma_start(wqr2T_sb[:], wqr2T[:, :])
            nc.sync.dma_start(cos2d_sb[:], cos2d[:])
            nc.scalar.dma_start(sin2d_sb[:], sin2d[:])
            nc.scalar.dma_start(mask_r[:], maskp[:])
            nc.scalar.dma_start(perm_sb[:], permp[:])

            # ---------- phase 1: unpack gathered latents ----------
            # cqT_f/ckvT_f are r-major: col = ts*2048 + qc*512 + t_loc, so
            # each rank's block is ONE contiguous DMA from the gather output
            # split unpack across both HWDGE rings (SP + ACT) for parallelism
            ag1 = agout1.ap() if sim_single else agout1[:]
            ag2 = agout2.ap() if sim_single else agout2[:]
            ag3 = None
            if FP8Q:
                ag3 = agout3.ap() if sim_single else agout3[:]
            for r in range(2):
                rows = slice(r * 128, (r + 1) * 128)
                eng = nc.sync if r == 0 else nc.scalar
                if FP8KV:
                    eng.dma_start(ckvT_f[:, r * 4:(r + 1) * 4, :],
                                  ag1[rows, :])
                else:
                    eng.dma_start(
                        ckvT_f[:, r * 4 * TOK:(r + 1) * 4 * TOK],
                        ag1[rows, :])
                if FP8Q:
                    eng.dma_start(cqT_f[:, r * 4:(r + 1) * 4, :],
                                  ag2[rows, :])
                    eng.dma_start(
                        krT_f[:, r * TOK:(r + 1) * TOK],
                        ag3[r * 128: r * 128 + DHR, :])
                else:
                    eng.dma_start(
                        cqT_f[:, r * 4 * TOK:(r + 1) * 4 * TOK],
                        ag2[rows, :4 * TOK])
                    eng.dma_start(
                        krT_f[:, r * TOK:(r + 1) * TOK],
                        ag2[r * 128: r * 128 + DHR, 4 * TOK:])

            # ---------- phase 2: per-head projections + attention ----------
            with (
                tc.tile_pool(name="pv2", bufs=1) as pv2,
                tc.tile_pool(name="ph", bufs=2) as ph,
                tc.tile_pool(name="pqr", bufs=2) as pqr,
                tc.tile_pool(name="pex", bufs=14) as pex,
                tc.tile_pool(name="py", bufs=2) as py,
                tc.For_i(0, loop_k, 1, hint_engines=(mybir.EngineType.PE,))
                if loop_k else contextlib.nullcontext(),
            ):
                hpc_eff = 0 if phases == "pre" else HPC
                for tok in variant.split(","):
                    if tok.startswith("h") and tok[1:].isdigit():
                        hpc_eff = int(tok[1:])
                D2 = HPC * HS
                # softmax denominators: per head one PSUM bank with th=0 at
                # partition 0 and th=1 at partition 64 (both groups
                # accumulate concurrently); all 16 rows gathered in SBUF and
                # written out once
                if hpc_eff and not oldsums:
                    sums_sb = pv2.tile([1, 16 * 512], f32, tag="sums_sb")
                # v_s[s-chunk, d] for all 8 heads: [128, 8sc x (8h x 128d)]
                if hpc_eff:
                    v_s = pv2.tile([128, 8 * HPC * HS], bf16, tag="v_s")
                for sc in range(8 if hpc_eff else 0):
                    scb = (sc // 4) * 2048 + (sc % 4) * 128
                    pv = ps_work.tile([128, T], f32, tag="work")
                    # kc outer so the stationary ckv chunk is loaded once
                    # per (sc, kc) and reused by both nn matmuls
                    ts_, s_off = sc // 4, (sc % 4) * 128
                    for kc in range(2 if FP8KV else 4):
                        for nn in range(2):
                            if FP8KV:
                                nc.tensor.matmul(
                                    pv[:, nn * 512:(nn + 1) * 512],
                                    ckvT_f[:, ts_ * 4 + 2 * kc: ts_ * 4 + 2 * kc + 2,
                                           s_off:s_off + 128],
                                    v_eff_sb[:, 2 * kc:2 * kc + 2,
                                             nn * 512:(nn + 1) * 512],
                                    start=(kc == 0), stop=(kc == 1),
                                    perf_mode=DR)
                            else:
                                nc.tensor.matmul(
                                    pv[:, nn * 512:(nn + 1) * 512],
                                    ckvT_f[:, scb + kc * 512: scb + kc * 512 + 128],
                                    v_eff_sb[:, kc * D2 + nn * 512: kc * D2 + (nn + 1) * 512],
                                    start=(kc == 0), stop=(kc == 3))
                    if sc % 2:
                        nc.vector.tensor_copy(v_s[:, sc * D2:(sc + 1) * D2],
                                              pv[:])
                    else:
                        nc.scalar.activation(v_s[:, sc * D2:(sc + 1) * D2],
                                             pv[:], Copy)

                for hh in range(hpc_eff):
                    # roped q_r for a PAIR of heads at once (tile rows:
                    # head-even 0:64, head-odd 64:128), split on evac.
                    # Issued FIRST so the PE->DVE->Pool rope chain finishes
                    # under the kh/qh matmuls (logits j=0 needs qr).
                    if hh % 2 == 0:
                        pp = hh // 2
                        qrA = pqr.tile([DHR, T], bf16, tag="qrA")
                        qrB = pqr.tile([DHR, T], bf16, tag="qrB")
                        qraw = pqr.tile([128, T], bf16, tag="qraw")
                        qrtmp2 = [pqr.tile([128, 2 * 512], bf16, tag="qrtmp",
                                           name=f"qrtmp_{hh}_{ts}")
                                  for ts in range(2)]
                        # raw projection only; the pair-swapped copy is a
                        # cheap 128x128 permutation matmul on the 128-dim
                        # output (issued after kh so the raw evac hides)
                        prw = [ps_work.tile([128, T], f32, tag="work",
                                            name=f"pr2_{hh}_{ts}")
                               for ts in range(2)]
                        for ts in range(2):
                            for qc in range(4):
                                nc.tensor.matmul(
                                    prw[ts][:, :512],
                                    wqr2T_sb[:, pp * 512 + qc * 128: pp * 512 + (qc + 1) * 128],
                                    cqT_f[:, ts * 2048 + qc * 512: ts * 2048 + (qc + 1) * 512],
                                    start=(qc == 0), stop=(qc == 3))
                            nc.vector.tensor_copy(
                                qraw[:, ts * 512:(ts + 1) * 512],
                                prw[ts][:, :512])
                            nc.vector.tensor_tensor(
                                qrtmp2[ts][:, :512],
                                qraw[:, ts * 512:(ts + 1) * 512],
                                cos2d_sb[:, ts * 512:(ts + 1) * 512], mult)
                    qr = qrA if hh % 2 == 0 else qrB

                    # q_hT[d, t], k_hT[d, s] for the full batch; qc outer /
                    # ts inner reuses each stationary weight chunk twice;
                    # evacs split across DVE/ACT so neither serializes the
                    # next projection group
                    qh = ph.tile([128, T], bf16, tag="qh")
                    kh = ph.tile([128, T], bf16, tag="kh")
                    for ts in range(2):
                        pk = ps_proj.tile([128, 512], f32, tag="proj",
                                          name=f"pk_{hh}_{ts}")
                        for kc in range(2 if FP8KV else 4):
                            if FP8KV:
                                nc.tensor.matmul(
                                    pk[:],
                                    wukT_sb[:, hh * 4 + 2 * kc: hh * 4 + 2 * kc + 2, :],
                                    ckvT_f[:, ts * 4 + 2 * kc: ts * 4 + 2 * kc + 2, :],
                                    start=(kc == 0), stop=(kc == 1),
                                    perf_mode=DR)
                            else:
                                nc.tensor.matmul(
                                    pk[:],
                                    wukT_sb[:, hh * 512 + kc * 128: hh * 512 + (kc + 1) * 128],
                                    ckvT_f[:, ts * 2048 + kc * 512: ts * 2048 + (kc + 1) * 512],
                                    start=(kc == 0), stop=(kc == 3))
                        nc.vector.tensor_copy(kh[:, ts * 512:(ts + 1) * 512],
                                              pk[:])
                    if hh % 2 == 0:
                        for ts in range(2):
                            nc.tensor.matmul(
                                prw[ts][:, 512:1024], perm_sb[:],
                                qraw[:, ts * 512:(ts + 1) * 512],
                                start=True, stop=True)
                        for ts in range(2):
                            tsl = slice(ts * 512, (ts + 1) * 512)
                            nc.vector.tensor_tensor(
                                qrtmp2[ts][:, 512:], prw[ts][:, 512:1024],
                                sin2d_sb[:, tsl], mult)
                            nc.gpsimd.tensor_tensor(
                                qrA[:, tsl], qrtmp2[ts][:DHR, :512],
                                qrtmp2[ts][:DHR, 512:], add)
                            nc.gpsimd.tensor_tensor(
                                qrB[:, tsl], qrtmp2[ts][DHR:, :512],
                                qrtmp2[ts][DHR:, 512:], add)
                    for ts in range(2):
                        pq = ps_proj.tile([128, 512], f32, tag="proj",
                                          name=f"pq_{hh}_{ts}")
                        for qc in range(2 if FP8Q else 4):
                            if FP8Q:
                                nc.tensor.matmul(
                                    pq[:],
                                    wuqT_sb[:, hh * 4 + 2 * qc: hh * 4 + 2 * qc + 2, :],
                                    cqT_f[:, ts * 4 + 2 * qc: ts * 4 + 2 * qc + 2, :],
                                    start=(qc == 0), stop=(qc == 1),
                                    perf_mode=DR)
                            else:
                                nc.tensor.matmul(
                                    pq[:],
                                    wuqT_sb[:, hh * 512 + qc * 128: hh * 512 + (qc + 1) * 128],
                                    cqT_f[:, ts * 2048 + qc * 512: ts * 2048 + (qc + 1) * 512],
                                    start=(qc == 0), stop=(qc == 3))
                        nc.vector.tensor_copy(qh[:, ts * 512:(ts + 1) * 512],
                                              pq[:])

                    y_sb = py.tile([128, T], bf16, tag="y")
                    if oldsums:
                        sums_sb = py.tile([1, T], f32, tag="sums")
                    # pass 1: logits + exp, 8 s-chunks over the full t-range
                    # (one [128, njt] exp per chunk; lg spans 2 PSUM banks).
                    # content pair first then rope pair so each stationary
                    # (kh / krT chunk) is loaded once.
                    exs = []
                    for j in range(8):
                        t_off = 128 * j
                        lg = ps_work.tile([128, T], f32, tag="work")
                        if "wide" in variant:
                            csls = [slice(t_off, T)]
                        else:
                            csls = []
                            for th in range(2):
                                c0 = max(t_off, th * 512)
                                if c0 >= (th + 1) * 512:
                                    continue
                                csls.append(slice(c0, (th + 1) * 512))
                        for csl in csls:
                            nc.tensor.matmul(
                                lg[:, csl], kh[:, j * 128:(j + 1) * 128],
                                qh[:, csl], start=True, stop=False)
                        for csl in csls:
                            nc.tensor.matmul(
                                lg[:, csl], krT_f[:, j * 128:(j + 1) * 128],
                                qr[:, csl], start=False, stop=True)
                        ex = pex.tile([128, T], bf16, tag="ex",
                                      name=f"ex_{hh}_{j}")
                        nc.scalar.activation(ex[:, t_off:], lg[:, t_off:],
                                             Exp, scale=SCALE)
                        nc.gpsimd.tensor_tensor(
                            ex[:, t_off:t_off + 128], ex[:, t_off:t_off + 128],
                            mask_r[:], mult)
                        exs.append(ex)
                    # pass 2: AV + sums accumulation per t-half bank
                    for th in range(2):
                        av_ps = ps_av.tile([128, 512], f32, tag="av")
                        if "nosums" not in variant:
                            sums_ps = ps_sums.tile([1, 512], f32, tag="sums",
                                                   name=f"sums_{hh}_{th}")
                        njs = [j for j in range(8) if 128 * j < (th + 1) * 512]
                        for i, j in enumerate(njs):
                            c0 = max(128 * j, th * 512)
                            esl = slice(c0, (th + 1) * 512)
                            osl = slice(c0 - th * 512, 512)
                            first, last = (i == 0), (i == len(njs) - 1)
                            nc.tensor.matmul(
                                av_ps[:, osl],
                                v_s[:, j * D2 + hh * HS: j * D2 + (hh + 1) * HS],
                                exs[j][:, esl], start=first, stop=last)
                        for i, j in enumerate(njs):
                            if "nosums" in variant:
                                break
                            c0 = max(128 * j, th * 512)
                            esl = slice(c0, (th + 1) * 512)
                            osl = slice(c0 - th * 512, 512)
                            first, last = (i == 0), (i == len(njs) - 1)
                            nc.tensor.matmul(
                                sums_ps[:, osl], ones_r[:],
                                exs[j][:, esl], start=first, stop=last)
                        tsl = slice(th * 512, (th + 1) * 512)
                        if th == 0:
                            nc.scalar.activation(y_sb[:, tsl], av_ps[:], Copy)
                        else:
                            nc.vector.tensor_copy(y_sb[:, tsl], av_ps[:])
                        if "nosums" in variant:
                            pass
                        elif oldsums:
                            nc.scalar.activation(sums_sb[:, tsl], sums_ps[:],
                                                 Copy)
                        else:
                            r = (2 * hh + th) * 512
                            nc.vector.tensor_copy(
                                sums_sb[:, r:r + 512], sums_ps[:])
                    nc.sync.dma_start(out[hh * HS:(hh + 1) * HS, :], y_sb[:])
                    if oldsums:
                        nc.sync.dma_start(
                            out2[:, 2 * hh * 512:(2 * hh + 1) * 512],
                            sums_sb[:1, :512])
                        nc.sync.dma_start(
                            out2[:, (2 * hh + 1) * 512:(2 * hh + 2) * 512],
                            sums_sb[:1, 512:])
                if hpc_eff and not oldsums and "nosums" not in variant:
                    nc.sync.dma_start(out2[:, :], sums_sb[:])
    nc.compile()
    return nc


def _pairswap(w):
    idx = np.arange(w.shape[0]).reshape(-1, 2)[:, ::-1].reshape(-1)
    return w[idx]


def _slab(m, dtype):
    """[n*128, W] row-major -> SBUF slab layout [128, n*W]."""
    n = m.shape[0] // 128
    return np.ascontiguousarray(
        m.reshape(n, 128, m.shape[1]).transpose(1, 0, 2).reshape(128, -1),
        dtype=dtype)


def _make_in_maps(x, W_dq, W_uq, W_dkv, W_uk, W_uv, W_o, W_qr, W_kr,
                  freqs_cos, freqs_sin):
    import ml_dtypes
    f4 = np.float32
    bf = ml_dtypes.bfloat16
    kv = ml_dtypes.float8_e4m3 if FP8KV else bf
    qd = ml_dtypes.float8_e4m3 if FP8Q else bf
    wdqT = _slab(W_dq.T, bf)
    wdkvT = _slab(W_dkv.T, bf)
    wkr2T = _slab(np.concatenate([W_kr.T, _pairswap(W_kr).T], axis=1), bf)
    veff_full = W_uv.T.astype(f4) @ W_o.T.astype(f4)   # (NLKV, C) on host
    uq = W_uq.reshape(NLQ, NH, HS)
    uk = W_uk.reshape(NH, HS, NLKV)
    cos2 = np.repeat(freqs_cos.T, 2, axis=0).astype(f4)          # [DHR, T]
    sin_half = freqs_sin.T.astype(f4)                            # [DHR/2, T]
    sin2 = np.empty((DHR, T), dtype=f4)
    sin2[0::2] = -sin_half
    sin2[1::2] = sin_half
    cos2d = np.concatenate([cos2, cos2], axis=0).astype(bf)      # [128, T]
    sin2d = np.concatenate([sin2, sin2], axis=0)
    perm = np.zeros((128, 128), dtype=bf)
    perm[np.arange(128), np.arange(128) ^ 1] = 1

    in_maps = []
    for i in range(NCORES):
        b_own, half = divmod(i, 2)
        t0 = half * TOK
        heads = [HPC * half + hh for hh in range(HPC)]
        # per-head lhsT slabs: block hh at cols [hh*512 + qc*128]
        wuqT = np.concatenate(
            [_slab(uq[:, h, :], qd) for h in heads], axis=1)     # A_h (NLQ, HS)
        wukT = np.concatenate(
            [_slab(uk[h].T, kv) for h in heads], axis=1)         # B_h.T (NLKV, HS)
        qr_tiles = []
        for p in range(HPC // 2):
            hA, hB = heads[2 * p], heads[2 * p + 1]
            t1 = np.concatenate([W_qr[hA * DHR:(hA + 1) * DHR].T,
                                 W_qr[hB * DHR:(hB + 1) * DHR].T], axis=1)
            qr_tiles += [t1]
        wqr2T = np.concatenate([_slab(t, bf) for t in qr_tiles], axis=1)
        veffp = _slab(veff_full[:, heads[0] * HS:(heads[-1] + 1) * HS], kv)
        in_maps.append({
            "xT": _slab(x[b_own, t0:t0 + TOK, :].T, bf),
            "wdqT": wdqT, "wdkvT": wdkvT, "wkr2T": wkr2T,
            "wuqT": wuqT, "wukT": wukT, "wqr2T": wqr2T,
            "veffp": veffp,
            "cos2d": cos2d, "sin2d": sin2d,
            "cos2o": np.ascontiguousarray(cos2[:, t0:t0 + TOK]),
            "sin2o": np.ascontiguousarray(sin2[:, t0:t0 + TOK]),
            "maskp": np.triu(np.ones((128, 128))).astype(bf),
            "permp": perm,
        })
    return in_maps


def _assemble(results):
    y = np.empty((B, T, C), dtype=np.float32)
    for i in range(NCORES):
        b_own, half = divmod(i, 2)
        o = np.asarray(results[i]["out"], dtype=np.float32)  # unnormalized
        s2 = results[i]["out2"].reshape(16, 512)  # row 2*hh+th = denoms
        for hh in range(HPC):
            h = HPC * half + hh
            den = np.concatenate([s2[2 * hh], s2[2 * hh + 1]])   # [T]
            blk = o[hh * HS:(hh + 1) * HS, :] / den
            y[b_own, :, h * HS:(h + 1) * HS] = blk.T
    return y


def kernel(**inputs):
    from concourse import bass_utils
    if "nc" not in _cache:
        _cache["nc"] = _build()
    nc = _cache["nc"]
    in_maps = _make_in_maps(**{k: np.asarray(v) for k, v in inputs.items()})
    res = bass_utils.run_bass_kernel_spmd(nc, in_maps, core_ids=list(range(NCORES)))
    return _assemble(res.results)
